# revision 18
# baseline (speedup 1.0000x reference)
"""Trainium2 Bass kernel: GQA attention block (QKV proj + RMSNorm + RoPE +
bidirectional attention + output proj), 8-way parallel.

Sharding: 8 cores = 4 batches x 2 query-token halves. K/V projection work is
deduplicated across the two cores of a batch: each core computes K/V for only
4 of the 8 kv heads (even core: kv 0-3, odd core: kv 4-7, selected by its
wkvT input slice) and the halves are exchanged with a pairwise AllGather
(replica groups [0,1],[2,3],[4,5],[6,7]), overlapped with the Q projection.
Each core then runs attention + o_proj for its 512 query tokens. Host gathers
the 8 output shards.

Per-core kernel (all matmuls in bf16, fp32 accumulation):
  P1  K proj (4 kv heads, all 1024 tokens) -> RMSNorm+RoPE -> PE-transpose
      -> DRAM -> AllGather -> ktT [d, kt, kv, t] (all 8 kv heads)
      V proj likewise (no norm/rope) -> v_all [t, tt, kv, d]
  P2  Q projection + attention, software-pipelined per head: chunk c+1's
      projection matmuls are interleaved with chunk c's score matmuls and
      the previous head's AV matmuls, so the PE never waits on the ScalarE
      exp tail or the GpSimd softmax-denominator reduction.
  P3  o_proj, a-major weight streaming: y [t, o] = aT.T @ woT, fp32 out
"""

import os
import sys
from contextlib import ExitStack

for _p in (
    "/root/.axon_site",
    "/root/.axon_site/_ro/trn_rl_repo",
    "/root/.axon_site/_ro/pypackages",
    "/opt/trn_rl_repo",
):
    if os.path.isdir(_p) and _p not in sys.path:
        sys.path.append(_p)

import ml_dtypes
import numpy as np

import concourse.bacc as bacc
import concourse.bass as bass
import concourse.tile as tile
from concourse import bass_isa, mybir
from concourse.bass_utils import run_bass_kernel_spmd
from concourse.masks import make_identity

BF16 = mybir.dt.bfloat16
F32 = mybir.dt.float32
AF = mybir.ActivationFunctionType
OP = mybir.AluOpType
AX = mybir.AxisListType

B = 4
S = 1024
SQ = 512            # query tokens per core
HIDDEN = 4096
NH = 32
NKV = 8
HD = 128
EPS = 1e-6
ROPE_BASE = 1000000.0
SCALE = float(HD) ** -0.5
NDT = HIDDEN // 128  # 32 contraction tiles
N_CORES = 8
PAIRS = [[0, 1], [2, 3], [4, 5], [6, 7]]

_BF = ml_dtypes.bfloat16


def _bcast_mid(ap, n):
    """[P, X...] -> [P, n, X...] with a stride-0 middle dim."""
    return bass.AP(tensor=ap.tensor, offset=ap.offset, ap=[ap.ap[0], [0, n], *ap.ap[1:]])


def build_bass() -> bass.Bass:
    nc = bacc.Bacc("TRN2", target_bir_lowering=False, debug=False, num_devices=N_CORES)

    # DRAM I/O (per core). hs blocks pre-arranged on host as [tile, p, a, t]
    # so each DMA is one contiguous 1MB read.
    hs_kv = nc.declare_dram_parameter("hs_kv", [8, 128, NDT, 128], BF16, isOutput=False)
    hs_q = nc.declare_dram_parameter("hs_q", [4, 128, NDT, 128], BF16, isOutput=False)
    wkvT = nc.declare_dram_parameter("wkvT", [HIDDEN, 1024], BF16, isOutput=False)
    wqT = nc.declare_dram_parameter("wqT", [HIDDEN, HIDDEN], BF16, isOutput=False)
    woT = nc.declare_dram_parameter("woT", [HIDDEN, HIDDEN], BF16, isOutput=False)
    # rope tables [t, cA|sA|cB|sB] (cos/sin with rms-norm weight folded in)
    ropeq = nc.declare_dram_parameter("ropeq", [SQ, 256], F32, isOutput=False)
    ropek = nc.declare_dram_parameter("ropek", [S, 256], F32, isOutput=False)
    y = nc.declare_dram_parameter("y", [SQ, HIDDEN], F32, isOutput=True)

    with ExitStack() as ctx:
        tc = ctx.enter_context(tile.TileContext(nc))

        persist = ctx.enter_context(tc.tile_pool(name="persist", bufs=1))
        ktT = persist.tile([128, 8, NKV, 128], BF16, tag="ktT")   # [d, kt, kvh, t]
        v_all = persist.tile([128, 8, NKV, 128], BF16, tag="v")   # [t%128, tt, kvh, d]
        aT = persist.tile([128, NH, SQ], BF16, tag="aT")          # [d, h, q]
        tabq = persist.tile([128, 4, 256], F32, tag="tabq")
        tabk = persist.tile([128, 8, 256], F32, tag="tabk")
        ident = persist.tile([128, 128], BF16, tag="ident")

        wp = ctx.enter_context(tc.tile_pool(name="wp", bufs=10))
        hp = ctx.enter_context(tc.tile_pool(name="hp", bufs=4))
        scratch = ctx.enter_context(tc.tile_pool(name="scratch", bufs=2))
        qtp = ctx.enter_context(tc.tile_pool(name="qtp", bufs=1))
        qnp = ctx.enter_context(tc.tile_pool(name="qnp", bufs=6))
        attn_sb = ctx.enter_context(tc.tile_pool(name="attn_sb", bufs=2))
        ysb = ctx.enter_context(tc.tile_pool(name="ysb", bufs=2))

        dram = ctx.enter_context(tc.tile_pool(name="dram", bufs=1, space="DRAM"))
        cc_kin = dram.tile([128, 8, 4, 128], BF16, tag="cc_kin")
        cc_kout = dram.tile([2, 128, 8, 4, 128], BF16, tag="cc_kout")
        cc_vin1 = dram.tile([128, 4, 4, 128], BF16, tag="cc_vin1")
        cc_vout1 = dram.tile([2, 128, 4, 4, 128], BF16, tag="cc_vout1")
        cc_vin2 = dram.tile([128, 4, 4, 128], BF16, tag="cc_vin2")
        cc_vout2 = dram.tile([2, 128, 4, 4, 128], BF16, tag="cc_vout2")

        pp_ps = ctx.enter_context(tc.tile_pool(name="pp_ps", bufs=3, space="PSUM"))
        st_ps = ctx.enter_context(tc.tile_pool(name="st_ps", bufs=3, space="PSUM"))
        av_ps = ctx.enter_context(tc.tile_pool(name="av_ps", bufs=2, space="PSUM"))

        def load_w_group(wsrc, col0, g):
            """One [128, 4, 512] tile covering rows g*512..g*512+512 (4
            contraction tiles), cols col0:col0+512 — a single 512KB DMA, so
            4 consecutive matmuls share one producer dependency."""
            wt = wp.tile([128, 4, 512], BF16, tag="wt")
            nc.sync.dma_start(
                out=wt[:],
                in_=wsrc[g * 512:(g + 1) * 512, col0:col0 + 512].rearrange(
                    "(a p) c -> p a c", p=128),
            )
            return wt

        def load_w_tiles(wsrc, col0):
            return [load_w_group(wsrc, col0, g) for g in range(8)]

        def wslice(wts, a):
            return wts[a // 4][:, a % 4, :]

        def load_hs(src):
            """[128, 32, 128] hidden-state tile, split into 4 sub-DMAs so
            early matmuls only wait on their own quarter."""
            hs_cb = hp.tile([128, NDT, 128], BF16, tag="hs")
            for part in range(4):
                nc.sync.dma_start(out=hs_cb[:, part * 8:(part + 1) * 8, :],
                                  in_=src[:, part * 8:(part + 1) * 8, :])
            return hs_cb

        # First weight group + first hs quarters first: nothing blocks the
        # PE longer than these at kernel start.
        wts_k = [load_w_group(wkvT, 0, 0)]
        hs_first = load_hs(hs_kv[0])
        for g in range(1, 8):
            wts_k.append(load_w_group(wkvT, 0, g))
        make_identity(nc, ident[:])
        nc.sync.dma_start(out=tabq[:], in_=ropeq[:].rearrange("(a p) c -> p a c", p=128))
        nc.sync.dma_start(out=tabk[:], in_=ropek[:].rearrange("(a p) c -> p a c", p=128))

        def norm_rope(ps, tab_tile, tt, qn):
            """RMSNorm + RoPE on a [128 tok, 4 heads, 128] psum projection,
            into bf16 qn [128, 4, 128]."""
            psv = ps[:].rearrange("p (h d) -> p h d", h=4)
            qf = scratch.tile([128, 4, 128], F32, tag="qf")
            qsq = scratch.tile([128, 512], BF16, tag="qsq", bufs=1)
            ssq = scratch.tile([128, 4], F32, tag="ssq")
            rr = scratch.tile([128, 4], F32, tag="rr")
            t1 = scratch.tile([128, 4, 64], F32, tag="tq")
            t2 = scratch.tile([128, 4, 64], F32, tag="tq")
            t3 = scratch.tile([128, 4, 64], F32, tag="tq", name="t3")
            t4 = scratch.tile([128, 4, 64], F32, tag="tq", name="t4")

            nc.vector.tensor_scalar_mul(qf[:], psv, 1.0)
            nc.vector.tensor_mul(qsq[:], qf[:].rearrange("p h d -> p (h d)"), ps[:])
            nc.vector.reduce_sum(
                out=ssq[:], in_=qsq[:].rearrange("p (h d) -> p h d", h=4), axis=AX.X
            )
            # v = ssq/128 + eps, then r = rsqrt(v) via bit-trick seed + 2 Newton
            # iterations (all-DVE; keeps ScalarE on a single ACT table set).
            vv = scratch.tile([128, 4], F32, tag="vv")
            rt = scratch.tile([128, 4], F32, tag="rt")
            nc.vector.tensor_scalar(out=vv[:], in0=ssq[:], scalar1=1.0 / HD,
                                    scalar2=EPS, op0=OP.mult, op1=OP.add)
            vi = vv[:].bitcast(mybir.dt.int32)
            ri = rr[:].bitcast(mybir.dt.int32)
            nc.vector.tensor_scalar(out=ri, in0=vi, scalar1=1, scalar2=None,
                                    op0=OP.arith_shift_right)
            nc.vector.tensor_scalar(out=ri, in0=ri, scalar1=-1, scalar2=0x5F3759DF,
                                    op0=OP.mult, op1=OP.add)
            for _ in range(2):
                nc.vector.tensor_mul(rt[:], rr[:], rr[:])
                nc.vector.tensor_mul(rt[:], rt[:], vv[:])
                nc.vector.tensor_scalar(out=rt[:], in0=rt[:], scalar1=-0.5,
                                        scalar2=1.5, op0=OP.mult, op1=OP.add)
                nc.vector.tensor_mul(rr[:], rr[:], rt[:])
            for hh in range(4):
                nc.vector.tensor_scalar_mul(qf[:, hh, :], qf[:, hh, :], rr[:, hh:hh + 1])
            q1 = qf[:, :, 0:64]
            q2 = qf[:, :, 64:128]
            cA = _bcast_mid(tab_tile[:, tt, 0:64], 4)
            sA = _bcast_mid(tab_tile[:, tt, 64:128], 4)
            cB = _bcast_mid(tab_tile[:, tt, 128:192], 4)
            sB = _bcast_mid(tab_tile[:, tt, 192:256], 4)
            nc.vector.tensor_mul(t1[:], q1, cA)
            nc.vector.tensor_mul(t2[:], q2, sB)
            nc.vector.tensor_sub(qn[:, :, 0:64], t1[:], t2[:])
            nc.vector.tensor_mul(t3[:], q2, cB)
            nc.vector.tensor_mul(t4[:], q1, sA)
            nc.vector.tensor_add(qn[:, :, 64:128], t3[:], t4[:])

        def transpose4(qn, dst_ap):
            """PE-transpose 4 [128,128] heads of qn into dst_ap [128, 4, 128]."""
            tp = st_ps.tile([128, 512], BF16, tag="misc")
            for hh in range(4):
                nc.tensor.transpose(tp[:, hh * 128:(hh + 1) * 128], qn[:, hh, :], ident[:])
            nc.scalar.copy(out=dst_ap, in_=tp[:].rearrange("p (h t) -> p h t", h=4))

        # ---------------- P1: K then V projection (own 4 kv heads) -----
        # K transposes are deferred one tile behind the matmul stream so
        # the PE never waits for the DVE norm/rope tail. The K exchange is
        # split in two so the first AllGather fires ~halfway through the K
        # pass; bounce-buffer DMAs ride the ScalarE queue (SyncE backlogs).
        def flush_k(kn, tt):
            ktn = qnp.tile([128, 4, 128], BF16, tag="qqn", name="ktn")
            transpose4(kn, ktn[:])
            nc.scalar.dma_start(out=cc_kin[:, tt], in_=ktn[:])

        pend_k = None
        for tt in range(8):
            hs_cb = hs_first if tt == 0 else load_hs(hs_kv[tt])
            ps = pp_ps.tile([128, 512], F32, tag="pp")
            for a in range(NDT):
                nc.tensor.matmul(
                    ps[:], hs_cb[:, a, :], wslice(wts_k, a),
                    start=(a == 0), stop=(a == NDT - 1),
                )
            kn = qnp.tile([128, 4, 128], BF16, tag="qqn")
            norm_rope(ps, tabk, tt, kn)
            if pend_k is not None:
                flush_k(*pend_k)
            pend_k = (kn, tt)
        flush_k(*pend_k)
        nc.gpsimd.collective_compute(
            "AllGather", OP.bypass, replica_groups=PAIRS,
            ins=[cc_kin[:]], outs=[cc_kout[:]],
        )
        for half in range(2):
            ts = slice(half * 4, half * 4 + 4)
            nc.sync.dma_start(out=ktT[:, ts, 0:4, :], in_=cc_kout[0][:, ts])
            nc.sync.dma_start(out=ktT[:, ts, 4:8, :], in_=cc_kout[1][:, ts])

        wts_v = load_w_tiles(wkvT, 512)
        for tt in range(8):
            hs_cb = load_hs(hs_kv[tt])
            ps = pp_ps.tile([128, 512], F32, tag="pp")
            for a in range(NDT):
                nc.tensor.matmul(
                    ps[:], hs_cb[:, a, :], wslice(wts_v, a),
                    start=(a == 0), stop=(a == NDT - 1),
                )
            vn = qnp.tile([128, 4, 128], BF16, tag="qqn", name="vn")
            nc.scalar.copy(out=vn[:], in_=ps[:].rearrange("p (h d) -> p h d", h=4))
            cc_vdst = cc_vin1 if tt < 4 else cc_vin2
            nc.scalar.dma_start(out=cc_vdst[:, tt % 4], in_=vn[:])
            if tt == 3 or tt == 7:
                cc_in, cc_out = (cc_vin1, cc_vout1) if tt == 3 else (cc_vin2, cc_vout2)
                nc.gpsimd.collective_compute(
                    "AllGather", OP.bypass, replica_groups=PAIRS,
                    ins=[cc_in[:]], outs=[cc_out[:]],
                )
                ts = slice(0, 4) if tt == 3 else slice(4, 8)
                nc.sync.dma_start(out=v_all[:, ts, 0:4, :], in_=cc_out[0])
                nc.sync.dma_start(out=v_all[:, ts, 4:8, :], in_=cc_out[1])

        # ---------------- P2: Q projection + attention, pipelined ------
        # Per chunk c (4 q heads sharing kv head c): chunk c+1's
        # projection matmuls interleave with chunk c's score matmuls and
        # older heads' AV matmuls (a 2-deep pending queue), so exp
        # (ScalarE) and the softmax denominator (DVE+GpSimd) latency never
        # stalls the PE. Projections run as token-tile PAIRS sharing each
        # weight tile between two back-to-back matmuls: the second issue
        # skips the per-tile sync overhead the PE pays on a fresh tile.
        def emit_proj_pair(qt0, wts):
            hs0 = load_hs(hs_q[qt0])
            hs1 = load_hs(hs_q[qt0 + 1])
            ps0 = pp_ps.tile([128, 512], F32, tag="pp")
            ps1 = pp_ps.tile([128, 512], F32, tag="pp")
            for a in range(NDT):
                nc.tensor.matmul(ps0[:], hs0[:, a, :], wslice(wts, a),
                                 start=(a == 0), stop=(a == NDT - 1))
                nc.tensor.matmul(ps1[:], hs1[:, a, :], wslice(wts, a),
                                 start=(a == 0), stop=(a == NDT - 1))
            qn0 = qnp.tile([128, 4, 128], BF16, tag="qqn")
            norm_rope(ps0, tabq, qt0, qn0)
            qn1 = qnp.tile([128, 4, 128], BF16, tag="qqn")
            norm_rope(ps1, tabq, qt0 + 1, qn1)
            return [qn0, qn1]

        def emit_scores(c, qt, qTc):
            p_sb = attn_sb.tile([128, 8, 512], BF16, tag="p_sb", bufs=3)
            for kt in range(8):
                st = st_ps.tile([128, 512], F32, tag="misc")
                nc.tensor.matmul(
                    st[:], ktT[:, kt, c, :], qTc[:, qt, :],
                    start=True, stop=True,
                )
                nc.scalar.activation(out=p_sb[:, kt, :], in_=st[:],
                                     func=AF.Exp, scale=SCALE)
            # softmax denominator: sum p over the 8 key blocks (DVE), then
            # across partitions (GpSimd all-reduce), then 1/Z (DVE approx)
            acc = attn_sb.tile([128, 512], BF16, tag="acc")
            s01 = attn_sb.tile([128, 512], BF16, tag="s01", bufs=1)
            s23 = attn_sb.tile([128, 512], BF16, tag="s23", bufs=1)
            nc.vector.tensor_add(s01[:], p_sb[:, 0, :], p_sb[:, 1, :])
            nc.vector.tensor_add(s23[:], p_sb[:, 2, :], p_sb[:, 3, :])
            nc.vector.tensor_add(s01[:], s01[:], s23[:])
            nc.vector.tensor_add(acc[:], p_sb[:, 4, :], p_sb[:, 5, :])
            nc.vector.tensor_add(s23[:], p_sb[:, 6, :], p_sb[:, 7, :])
            nc.vector.tensor_add(acc[:], acc[:], s23[:])
            nc.vector.tensor_add(acc[:], acc[:], s01[:])
            zbc = attn_sb.tile([128, 512], F32, tag="zbc", bufs=1)
            nc.gpsimd.partition_all_reduce(out_ap=zbc[:], in_ap=acc[:],
                                           channels=128,
                                           reduce_op=bass_isa.ReduceOp.add)
            rz = attn_sb.tile([128, 512], F32, tag="rz", bufs=2)
            nc.vector.reciprocal_approx_fast(out=rz[:], in_=zbc[:])
            return (c, qt, p_sb, rz)

        def emit_avchain(pend):
            c, qt, p_sb, rz = pend
            h = c * 4 + qt
            av = av_ps.tile([128, 512], F32, tag="av")
            for kt in range(8):
                nc.tensor.matmul(
                    av[:], v_all[:, kt, c, :], p_sb[:, kt, :],
                    start=(kt == 0), stop=(kt == 7),
                )
            nc.vector.tensor_mul(aT[:, h, :], av[:], rz[:])

        from collections import deque
        pend = deque()

        def emit_sc_av(c, qt, qTc):
            if len(pend) >= 2:
                emit_avchain(pend.popleft())
            pend.append(emit_scores(c, qt, qTc))

        wts_q = load_w_tiles(wqT, 0)
        qn_next = emit_proj_pair(0, wts_q) + emit_proj_pair(2, wts_q)
        for c in range(8):
            qns = qn_next
            qTc = qtp.tile([128, 4, SQ], BF16, tag="qTc")
            for qt in range(4):
                transpose4(qns[qt], qTc[:, :, qt * 128:(qt + 1) * 128])
            if c < 7:
                wts_q = load_w_tiles(wqT, (c + 1) * 512)
                qn_next = emit_proj_pair(0, wts_q)
            emit_sc_av(c, 0, qTc)
            emit_sc_av(c, 1, qTc)
            if c < 7:
                qn_next = qn_next + emit_proj_pair(2, wts_q)
            emit_sc_av(c, 2, qTc)
            emit_sc_av(c, 3, qTc)
        while pend:
            emit_avchain(pend.popleft())

        # ---------------- P3: o_proj, a-major weight streaming ----------
        # Each weight tile is read by 4 back-to-back matmuls (one per query
        # block) and retires immediately, so the wp ring gives a deep DMA
        # prefetch runway and weights stream continuously. PSUM tiles
        # rotate through the pp/av/st pools (6 banks) so no pool is closed
        # or opened (pool boundaries act as barriers) and chunk boundaries
        # never wait on the previous chunk's PSUM evictions.
        ops_pools = [pp_ps, pp_ps, pp_ps, av_ps, av_ps, st_ps, st_ps]
        ops_tags = ["pp", "pp", "pp", "av", "av", "misc", "misc"]
        ops_idx = 0
        for c in range(8):
            pss = []
            for i in range(4):
                pool = ops_pools[ops_idx % 7]
                pss.append(pool.tile([128, 512], F32, tag=ops_tags[ops_idx % 7],
                                     name=f"ops_{c}_{i}"))
                ops_idx += 1
            for a in range(NDT):
                if a % 4 == 0:
                    wt = load_w_group(woT, c * 512, a // 4)
                for qt in range(4):
                    nc.tensor.matmul(
                        pss[qt][:], aT[:, a, qt * 128:(qt + 1) * 128], wt[:, a % 4, :],
                        start=(a == 0), stop=(a == NDT - 1),
                    )
            for qt in range(4):
                yt = ysb.tile([128, 512], F32, tag="yt")
                if qt % 2 == 0:
                    nc.scalar.copy(out=yt[:], in_=pss[qt][:])
                else:
                    nc.vector.tensor_scalar_mul(yt[:], pss[qt][:], 1.0)
                nc.sync.dma_start(
                    out=y[qt * 128:(qt + 1) * 128, c * 512:(c + 1) * 512], in_=yt[:]
                )

    nc.finalize()
    return nc


def _prep_inputs(inputs):
    pos = np.asarray(inputs["positions"]).astype(np.int32)
    hs = np.asarray(inputs["hidden_states"], dtype=np.float32)
    wq = np.asarray(inputs["wq"], dtype=np.float32)
    wk = np.asarray(inputs["wk"], dtype=np.float32)
    wv = np.asarray(inputs["wv"], dtype=np.float32)
    wo = np.asarray(inputs["wo"], dtype=np.float32)
    qw = np.asarray(inputs["q_norm_w"], dtype=np.float32)
    kw = np.asarray(inputs["k_norm_w"], dtype=np.float32)

    half = HD // 2
    inv_freq = (
        1.0 / (ROPE_BASE ** (np.arange(0, half, dtype=np.float32) * 2.0 / HD))
    ).astype(np.float32)
    ang = pos.astype(np.float32)[:, None] * inv_freq[None, :]  # [S, 64]
    cos = np.cos(ang).astype(np.float32)
    sin = np.sin(ang).astype(np.float32)

    def tab(w):
        w1, w2 = w[:half][None, :], w[half:][None, :]
        return np.ascontiguousarray(
            np.concatenate([cos * w1, sin * w1, cos * w2, sin * w2], axis=1)
        ).astype(np.float32)  # [S, 256] = [cA|sA|cB|sB]

    tq = tab(qw)
    tk = tab(kw)

    wqT = np.ascontiguousarray(wq.T).astype(_BF)
    woT = np.ascontiguousarray(wo.T).astype(_BF)
    # per-half K/V weight slices: even cores own kv heads 0-3, odd 4-7
    wkvT_half = []
    for kvh in range(2):
        rows = slice(kvh * 512, (kvh + 1) * 512)
        wkvT_half.append(
            np.ascontiguousarray(np.concatenate([wk[rows], wv[rows]], axis=0).T).astype(_BF)
        )

    in_maps = []
    for core in range(N_CORES):
        b, qh = core // 2, core % 2
        hsb = np.ascontiguousarray(hs[b].T).astype(_BF)  # [4096, 1024]
        # [a*128+p, tt*128+t] -> [tt, p, a, t]
        hkv = np.ascontiguousarray(
            hsb.reshape(NDT, 128, 8, 128).transpose(2, 1, 0, 3)
        )
        hq = np.ascontiguousarray(
            hsb[:, qh * SQ:(qh + 1) * SQ].reshape(NDT, 128, 4, 128).transpose(2, 1, 0, 3)
        )
        in_maps.append(
            dict(
                hs_kv=hkv,
                hs_q=hq,
                wkvT=wkvT_half[qh],
                wqT=wqT,
                woT=woT,
                ropeq=np.ascontiguousarray(tq[qh * SQ:(qh + 1) * SQ]),
                ropek=tk,
            )
        )
    return in_maps


_NC_CACHE = {}


def _get_nc():
    if "nc" not in _NC_CACHE:
        _NC_CACHE["nc"] = build_bass()
    return _NC_CACHE["nc"]


def _run(inputs, **spmd_kwargs):
    nc = _get_nc()
    in_maps = _prep_inputs(inputs)
    res = run_bass_kernel_spmd(nc, in_maps, list(range(N_CORES)), **spmd_kwargs)
    out = np.empty((B, S, HIDDEN), dtype=np.float32)
    for core in range(N_CORES):
        b, qh = core // 2, core % 2
        out[b, qh * SQ:(qh + 1) * SQ, :] = res.results[core]["y"]
    return out, res


def kernel(**inputs) -> np.ndarray:
    out, _ = _run(inputs)
    return out


if __name__ == "__main__":
    nc = build_bass()
    print("built OK:", len(nc.m.functions[0].blocks), "blocks")


# revision 21
# speedup vs baseline: 1.0339x; 1.0339x over previous
"""Trainium2 Bass kernel: GQA attention block (QKV proj + RMSNorm + RoPE +
bidirectional attention + output proj), 8-way parallel.

Sharding: 8 cores = 4 batches x 2 query-token halves. K/V projection work is
deduplicated across the two cores of a batch: each core computes K/V for only
4 of the 8 kv heads (even core: kv 0-3, odd core: kv 4-7, selected by its
wkvT input slice) and the halves are exchanged with a pairwise AllGather
(replica groups [0,1],[2,3],[4,5],[6,7]), overlapped with the Q projection.
Each core then runs attention + o_proj for its 512 query tokens. Host gathers
the 8 output shards.

Per-core kernel (all matmuls in bf16, fp32 accumulation):
  P1  K proj (4 kv heads, all 1024 tokens) -> RMSNorm+RoPE -> PE-transpose
      -> DRAM -> AllGather -> ktT [d, kt, kv, t] (all 8 kv heads)
      V proj likewise (no norm/rope) -> v_all [t, tt, kv, d]
  P2  Q projection + attention, software-pipelined per head: chunk c+1's
      projection matmuls are interleaved with chunk c's score matmuls and
      the previous head's AV matmuls, so the PE never waits on the ScalarE
      exp tail or the GpSimd softmax-denominator reduction.
  P3  o_proj, a-major weight streaming: y [t, o] = aT.T @ woT, fp32 out
"""

import os
import sys
from contextlib import ExitStack

for _p in (
    "/root/.axon_site",
    "/root/.axon_site/_ro/trn_rl_repo",
    "/root/.axon_site/_ro/pypackages",
    "/opt/trn_rl_repo",
):
    if os.path.isdir(_p) and _p not in sys.path:
        sys.path.append(_p)

import ml_dtypes
import numpy as np

import concourse.bacc as bacc
import concourse.bass as bass
import concourse.tile as tile
from concourse import bass_isa, mybir
from concourse.bass_utils import run_bass_kernel_spmd
from concourse.masks import make_identity

BF16 = mybir.dt.bfloat16
F32 = mybir.dt.float32
AF = mybir.ActivationFunctionType
OP = mybir.AluOpType
AX = mybir.AxisListType

B = 4
S = 1024
SQ = 512            # query tokens per core
HIDDEN = 4096
NH = 32
NKV = 8
HD = 128
EPS = 1e-6
ROPE_BASE = 1000000.0
SCALE = float(HD) ** -0.5
NDT = HIDDEN // 128  # 32 contraction tiles
N_CORES = 8
PAIRS = [[0, 1], [2, 3], [4, 5], [6, 7]]

_BF = ml_dtypes.bfloat16


def _bcast_mid(ap, n):
    """[P, X...] -> [P, n, X...] with a stride-0 middle dim."""
    return bass.AP(tensor=ap.tensor, offset=ap.offset, ap=[ap.ap[0], [0, n], *ap.ap[1:]])


def build_bass() -> bass.Bass:
    nc = bacc.Bacc("TRN2", target_bir_lowering=False, debug=False, num_devices=N_CORES)

    # DRAM I/O (per core). hs blocks pre-arranged on host as [tile, p, a, t]
    # so each DMA is one contiguous 1MB read.
    hs_kv = nc.declare_dram_parameter("hs_kv", [8, 128, NDT, 128], BF16, isOutput=False)
    hs_q = nc.declare_dram_parameter("hs_q", [4, 128, NDT, 128], BF16, isOutput=False)
    wkvT = nc.declare_dram_parameter("wkvT", [HIDDEN, 1024], BF16, isOutput=False)
    wqT = nc.declare_dram_parameter("wqT", [HIDDEN, HIDDEN], BF16, isOutput=False)
    woT = nc.declare_dram_parameter("woT", [HIDDEN, HIDDEN], BF16, isOutput=False)
    # rope tables [t, cA|sA|cB|sB] (cos/sin with rms-norm weight folded in)
    ropeq = nc.declare_dram_parameter("ropeq", [SQ, 256], F32, isOutput=False)
    ropek = nc.declare_dram_parameter("ropek", [S, 256], F32, isOutput=False)
    y = nc.declare_dram_parameter("y", [SQ, HIDDEN], F32, isOutput=True)

    with ExitStack() as ctx:
        tc = ctx.enter_context(tile.TileContext(nc))

        persist = ctx.enter_context(tc.tile_pool(name="persist", bufs=1))
        ktT = persist.tile([128, 8, NKV, 128], BF16, tag="ktT")   # [d, kt, kvh, t]
        v_all = persist.tile([128, 8, NKV, 128], BF16, tag="v")   # [t%128, tt, kvh, d]
        aT = persist.tile([128, NH, SQ], BF16, tag="aT")          # [d, h, q]
        tabq = persist.tile([128, 4, 256], F32, tag="tabq")
        tabk = persist.tile([128, 8, 256], F32, tag="tabk")
        ident = persist.tile([128, 128], BF16, tag="ident")

        wp = ctx.enter_context(tc.tile_pool(name="wp", bufs=11))
        hp = ctx.enter_context(tc.tile_pool(name="hp", bufs=3))
        scratch = ctx.enter_context(tc.tile_pool(name="scratch", bufs=2))
        qtp = ctx.enter_context(tc.tile_pool(name="qtp", bufs=1))
        qnp = ctx.enter_context(tc.tile_pool(name="qnp", bufs=6))
        attn_sb = ctx.enter_context(tc.tile_pool(name="attn_sb", bufs=2))
        ysb = ctx.enter_context(tc.tile_pool(name="ysb", bufs=2))

        dram = ctx.enter_context(tc.tile_pool(name="dram", bufs=1, space="DRAM"))
        cc_kin = dram.tile([128, 8, 4, 128], BF16, tag="cc_kin")
        cc_kout = dram.tile([2, 128, 8, 4, 128], BF16, tag="cc_kout")
        cc_vin1 = dram.tile([128, 4, 4, 128], BF16, tag="cc_vin1")
        cc_vout1 = dram.tile([2, 128, 4, 4, 128], BF16, tag="cc_vout1")
        cc_vin2 = dram.tile([128, 4, 4, 128], BF16, tag="cc_vin2")
        cc_vout2 = dram.tile([2, 128, 4, 4, 128], BF16, tag="cc_vout2")

        pp_ps = ctx.enter_context(tc.tile_pool(name="pp_ps", bufs=3, space="PSUM"))
        st_ps = ctx.enter_context(tc.tile_pool(name="st_ps", bufs=3, space="PSUM"))
        av_ps = ctx.enter_context(tc.tile_pool(name="av_ps", bufs=2, space="PSUM"))

        def load_w_group(wsrc, col0, g):
            """One [128, 4, 512] tile covering rows g*512..g*512+512 (4
            contraction tiles), cols col0:col0+512 — a single 512KB DMA, so
            4 consecutive matmuls share one producer dependency."""
            wt = wp.tile([128, 4, 512], BF16, tag="wt")
            nc.sync.dma_start(
                out=wt[:],
                in_=wsrc[g * 512:(g + 1) * 512, col0:col0 + 512].rearrange(
                    "(a p) c -> p a c", p=128),
            )
            return wt

        def load_w_tiles(wsrc, col0):
            return [load_w_group(wsrc, col0, g) for g in range(8)]

        def wslice(wts, a):
            return wts[a // 4][:, a % 4, :]

        def load_hs(src):
            """[128, 32, 128] hidden-state tile, split into 4 sub-DMAs so
            early matmuls only wait on their own quarter."""
            hs_cb = hp.tile([128, NDT, 128], BF16, tag="hs")
            for part in range(4):
                nc.sync.dma_start(out=hs_cb[:, part * 8:(part + 1) * 8, :],
                                  in_=src[:, part * 8:(part + 1) * 8, :])
            return hs_cb

        # First weight group + first hs quarters first: nothing blocks the
        # PE longer than these at kernel start.
        wts_k = [load_w_group(wkvT, 0, 0)]
        hs_first = load_hs(hs_kv[0])
        for g in range(1, 8):
            wts_k.append(load_w_group(wkvT, 0, g))
        make_identity(nc, ident[:])
        nc.sync.dma_start(out=tabq[:], in_=ropeq[:].rearrange("(a p) c -> p a c", p=128))
        nc.sync.dma_start(out=tabk[:], in_=ropek[:].rearrange("(a p) c -> p a c", p=128))

        def norm_rope(ps, tab_tile, tt, qn):
            """RMSNorm + RoPE on a [128 tok, 4 heads, 128] psum projection,
            into bf16 qn [128, 4, 128]."""
            psv = ps[:].rearrange("p (h d) -> p h d", h=4)
            qf = scratch.tile([128, 4, 128], F32, tag="qf")
            qsq = scratch.tile([128, 512], BF16, tag="qsq", bufs=1)
            ssq = scratch.tile([128, 4], F32, tag="ssq")
            rr = scratch.tile([128, 4], F32, tag="rr")
            t1 = scratch.tile([128, 4, 64], F32, tag="tq")
            t2 = scratch.tile([128, 4, 64], F32, tag="tq")
            t3 = scratch.tile([128, 4, 64], F32, tag="tq", name="t3")
            t4 = scratch.tile([128, 4, 64], F32, tag="tq", name="t4")

            nc.scalar.copy(out=qf[:], in_=psv)
            nc.scalar.activation(out=qsq[:], in_=ps[:], func=AF.Square)
            nc.vector.reduce_sum(
                out=ssq[:], in_=qsq[:].rearrange("p (h d) -> p h d", h=4), axis=AX.X
            )
            # v = ssq/128 + eps, then r = rsqrt(v) via bit-trick seed + 2 Newton
            # iterations (all-DVE; keeps ScalarE on a single ACT table set).
            vv = scratch.tile([128, 4], F32, tag="vv")
            rt = scratch.tile([128, 4], F32, tag="rt")
            nc.vector.tensor_scalar(out=vv[:], in0=ssq[:], scalar1=1.0 / HD,
                                    scalar2=EPS, op0=OP.mult, op1=OP.add)
            vi = vv[:].bitcast(mybir.dt.int32)
            ri = rr[:].bitcast(mybir.dt.int32)
            nc.vector.tensor_scalar(out=ri, in0=vi, scalar1=1, scalar2=None,
                                    op0=OP.arith_shift_right)
            nc.vector.tensor_scalar(out=ri, in0=ri, scalar1=-1, scalar2=0x5F3759DF,
                                    op0=OP.mult, op1=OP.add)
            for _ in range(2):
                nc.vector.tensor_mul(rt[:], rr[:], rr[:])
                nc.vector.tensor_mul(rt[:], rt[:], vv[:])
                nc.vector.tensor_scalar(out=rt[:], in0=rt[:], scalar1=-0.5,
                                        scalar2=1.5, op0=OP.mult, op1=OP.add)
                nc.vector.tensor_mul(rr[:], rr[:], rt[:])
            rv = rr[:]
            rr_b = bass.AP(tensor=rv.tensor, offset=rv.offset,
                           ap=[rv.ap[0], rv.ap[1], [0, 128]])
            nc.vector.tensor_mul(qf[:], qf[:], rr_b)
            q1 = qf[:, :, 0:64]
            q2 = qf[:, :, 64:128]
            cA = _bcast_mid(tab_tile[:, tt, 0:64], 4)
            sA = _bcast_mid(tab_tile[:, tt, 64:128], 4)
            cB = _bcast_mid(tab_tile[:, tt, 128:192], 4)
            sB = _bcast_mid(tab_tile[:, tt, 192:256], 4)
            nc.vector.tensor_mul(t1[:], q1, cA)
            nc.vector.tensor_mul(t2[:], q2, sB)
            nc.vector.tensor_sub(qn[:, :, 0:64], t1[:], t2[:])
            nc.vector.tensor_mul(t3[:], q2, cB)
            nc.vector.tensor_mul(t4[:], q1, sA)
            nc.vector.tensor_add(qn[:, :, 64:128], t3[:], t4[:])

        def transpose4(qn, dst_ap):
            """PE-transpose 4 [128,128] heads of qn into dst_ap [128, 4, 128]."""
            tp = st_ps.tile([128, 512], BF16, tag="misc")
            for hh in range(4):
                nc.tensor.transpose(tp[:, hh * 128:(hh + 1) * 128], qn[:, hh, :], ident[:])
            nc.scalar.copy(out=dst_ap, in_=tp[:].rearrange("p (h t) -> p h t", h=4))

        # ---------------- P1: K then V projection (own 4 kv heads) -----
        # K transposes are deferred one tile behind the matmul stream so
        # the PE never waits for the DVE norm/rope tail. The K exchange is
        # split in two so the first AllGather fires ~halfway through the K
        # pass; bounce-buffer DMAs ride the ScalarE queue (SyncE backlogs).
        def flush_k(kn, tt):
            ktn = qnp.tile([128, 4, 128], BF16, tag="qqn", name="ktn")
            transpose4(kn, ktn[:])
            nc.scalar.dma_start(out=cc_kin[:, tt], in_=ktn[:])

        pend_k = None
        for tt in range(8):
            hs_cb = hs_first if tt == 0 else load_hs(hs_kv[tt])
            ps = pp_ps.tile([128, 512], F32, tag="pp")
            for a in range(NDT):
                nc.tensor.matmul(
                    ps[:], hs_cb[:, a, :], wslice(wts_k, a),
                    start=(a == 0), stop=(a == NDT - 1),
                )
            kn = qnp.tile([128, 4, 128], BF16, tag="qqn")
            norm_rope(ps, tabk, tt, kn)
            if pend_k is not None:
                flush_k(*pend_k)
            pend_k = (kn, tt)
        flush_k(*pend_k)
        nc.gpsimd.collective_compute(
            "AllGather", OP.bypass, replica_groups=PAIRS,
            ins=[cc_kin[:]], outs=[cc_kout[:]],
        )
        for half in range(2):
            ts = slice(half * 4, half * 4 + 4)
            nc.gpsimd.dma_start(out=ktT[:, ts, 0:4, :], in_=cc_kout[0][:, ts])
            nc.gpsimd.dma_start(out=ktT[:, ts, 4:8, :], in_=cc_kout[1][:, ts])

        wts_v = load_w_tiles(wkvT, 512)
        wts_q0 = None
        for tt in range(8):
            hs_cb = load_hs(hs_kv[tt])
            if tt == 6:
                wts_q0 = load_w_tiles(wqT, 0)
            ps = pp_ps.tile([128, 512], F32, tag="pp")
            for a in range(NDT):
                nc.tensor.matmul(
                    ps[:], hs_cb[:, a, :], wslice(wts_v, a),
                    start=(a == 0), stop=(a == NDT - 1),
                )
            vn = qnp.tile([128, 4, 128], BF16, tag="vn", name="vn", bufs=2)
            nc.scalar.copy(out=vn[:], in_=ps[:].rearrange("p (h d) -> p h d", h=4))
            cc_vdst = cc_vin1 if tt < 4 else cc_vin2
            nc.scalar.dma_start(out=cc_vdst[:, tt % 4], in_=vn[:])
            if tt == 3 or tt == 7:
                cc_in, cc_out = (cc_vin1, cc_vout1) if tt == 3 else (cc_vin2, cc_vout2)
                nc.gpsimd.collective_compute(
                    "AllGather", OP.bypass, replica_groups=PAIRS,
                    ins=[cc_in[:]], outs=[cc_out[:]],
                )
                ts = slice(0, 4) if tt == 3 else slice(4, 8)
                nc.gpsimd.dma_start(out=v_all[:, ts, 0:4, :], in_=cc_out[0])
                nc.gpsimd.dma_start(out=v_all[:, ts, 4:8, :], in_=cc_out[1])

        # ---------------- P2: Q projection + attention, pipelined ------
        # Per chunk c (4 q heads sharing kv head c): chunk c+1's
        # projection matmuls interleave with chunk c's score matmuls and
        # older heads' AV matmuls (a 2-deep pending queue), so exp
        # (ScalarE) and the softmax denominator (DVE+GpSimd) latency never
        # stalls the PE. Projections run as token-tile PAIRS sharing each
        # weight tile between two back-to-back matmuls: the second issue
        # skips the per-tile sync overhead the PE pays on a fresh tile.
        def emit_proj_pair(qt0, wts):
            hs0 = load_hs(hs_q[qt0])
            hs1 = load_hs(hs_q[qt0 + 1])
            ps0 = pp_ps.tile([128, 512], F32, tag="pp")
            ps1 = pp_ps.tile([128, 512], F32, tag="pp")
            for a in range(NDT):
                nc.tensor.matmul(ps0[:], hs0[:, a, :], wslice(wts, a),
                                 start=(a == 0), stop=(a == NDT - 1))
                nc.tensor.matmul(ps1[:], hs1[:, a, :], wslice(wts, a),
                                 start=(a == 0), stop=(a == NDT - 1))
            qn0 = qnp.tile([128, 4, 128], BF16, tag="qqn")
            norm_rope(ps0, tabq, qt0, qn0)
            qn1 = qnp.tile([128, 4, 128], BF16, tag="qqn")
            norm_rope(ps1, tabq, qt0 + 1, qn1)
            return [qn0, qn1]

        def emit_scores(c, qt, qTc):
            p_sb = attn_sb.tile([128, 8, 512], BF16, tag="p_sb", bufs=3)
            for kt in range(8):
                st = st_ps.tile([128, 512], F32, tag="misc")
                nc.tensor.matmul(
                    st[:], ktT[:, kt, c, :], qTc[:, qt, :],
                    start=True, stop=True,
                )
                nc.scalar.activation(out=p_sb[:, kt, :], in_=st[:],
                                     func=AF.Exp, scale=SCALE)
            # softmax denominator: sum p over the 8 key blocks (DVE), then
            # across partitions (GpSimd all-reduce), then 1/Z (DVE approx)
            acc = attn_sb.tile([128, 512], BF16, tag="acc")
            s01 = attn_sb.tile([128, 512], BF16, tag="s01", bufs=1)
            s23 = attn_sb.tile([128, 512], BF16, tag="s23", bufs=1)
            nc.vector.tensor_add(s01[:], p_sb[:, 0, :], p_sb[:, 1, :])
            nc.vector.tensor_add(s23[:], p_sb[:, 2, :], p_sb[:, 3, :])
            nc.vector.tensor_add(s01[:], s01[:], s23[:])
            nc.vector.tensor_add(acc[:], p_sb[:, 4, :], p_sb[:, 5, :])
            nc.vector.tensor_add(s23[:], p_sb[:, 6, :], p_sb[:, 7, :])
            nc.vector.tensor_add(acc[:], acc[:], s23[:])
            nc.vector.tensor_add(acc[:], acc[:], s01[:])
            zbc = attn_sb.tile([128, 512], F32, tag="zbc", bufs=1)
            nc.gpsimd.partition_all_reduce(out_ap=zbc[:], in_ap=acc[:],
                                           channels=128,
                                           reduce_op=bass_isa.ReduceOp.add)
            rz = attn_sb.tile([128, 512], F32, tag="rz", bufs=2)
            nc.vector.reciprocal_approx_fast(out=rz[:], in_=zbc[:])
            return (c, qt, p_sb, rz)

        def emit_avchain(pend):
            c, qt, p_sb, rz = pend
            h = c * 4 + qt
            av = av_ps.tile([128, 512], F32, tag="av")
            for kt in range(8):
                nc.tensor.matmul(
                    av[:], v_all[:, kt, c, :], p_sb[:, kt, :],
                    start=(kt == 0), stop=(kt == 7),
                )
            nc.vector.tensor_mul(aT[:, h, :], av[:], rz[:])

        from collections import deque
        pend = deque()

        def emit_sc_av(c, qt, qTc):
            if len(pend) >= 2:
                emit_avchain(pend.popleft())
            pend.append(emit_scores(c, qt, qTc))

        wts_q = wts_q0
        qn_next = emit_proj_pair(0, wts_q) + emit_proj_pair(2, wts_q)
        for c in range(8):
            qns = qn_next
            qTc = qtp.tile([128, 4, SQ], BF16, tag="qTc")
            for qt in range(4):
                transpose4(qns[qt], qTc[:, :, qt * 128:(qt + 1) * 128])
            if c < 7:
                wts_q = load_w_tiles(wqT, (c + 1) * 512)
                qn_next = emit_proj_pair(0, wts_q)
            emit_sc_av(c, 0, qTc)
            emit_sc_av(c, 1, qTc)
            if c < 7:
                qn_next = qn_next + emit_proj_pair(2, wts_q)
            emit_sc_av(c, 2, qTc)
            emit_sc_av(c, 3, qTc)
        while pend:
            emit_avchain(pend.popleft())

        # ---------------- P3: o_proj, a-major weight streaming ----------
        # Each weight tile is read by 4 back-to-back matmuls (one per query
        # block) and retires immediately, so the wp ring gives a deep DMA
        # prefetch runway and weights stream continuously. PSUM tiles
        # rotate through the pp/av/st pools (6 banks) so no pool is closed
        # or opened (pool boundaries act as barriers) and chunk boundaries
        # never wait on the previous chunk's PSUM evictions.
        ops_pools = [pp_ps, pp_ps, pp_ps, av_ps, av_ps, st_ps, st_ps]
        ops_tags = ["pp", "pp", "pp", "av", "av", "misc", "misc"]
        ops_idx = 0
        for c in range(8):
            pss = []
            for i in range(4):
                pool = ops_pools[ops_idx % 7]
                pss.append(pool.tile([128, 512], F32, tag=ops_tags[ops_idx % 7],
                                     name=f"ops_{c}_{i}"))
                ops_idx += 1
            for a in range(NDT):
                if a % 4 == 0:
                    wt = load_w_group(woT, c * 512, a // 4)
                for qt in range(4):
                    nc.tensor.matmul(
                        pss[qt][:], aT[:, a, qt * 128:(qt + 1) * 128], wt[:, a % 4, :],
                        start=(a == 0), stop=(a == NDT - 1),
                    )
            for qt in range(4):
                yt = ysb.tile([128, 512], F32, tag="yt")
                if qt % 2 == 0:
                    nc.scalar.copy(out=yt[:], in_=pss[qt][:])
                else:
                    nc.vector.tensor_scalar_mul(yt[:], pss[qt][:], 1.0)
                nc.sync.dma_start(
                    out=y[qt * 128:(qt + 1) * 128, c * 512:(c + 1) * 512], in_=yt[:]
                )

    nc.finalize()
    return nc


def _prep_inputs(inputs):
    pos = np.asarray(inputs["positions"]).astype(np.int32)
    hs = np.asarray(inputs["hidden_states"], dtype=np.float32)
    wq = np.asarray(inputs["wq"], dtype=np.float32)
    wk = np.asarray(inputs["wk"], dtype=np.float32)
    wv = np.asarray(inputs["wv"], dtype=np.float32)
    wo = np.asarray(inputs["wo"], dtype=np.float32)
    qw = np.asarray(inputs["q_norm_w"], dtype=np.float32)
    kw = np.asarray(inputs["k_norm_w"], dtype=np.float32)

    half = HD // 2
    inv_freq = (
        1.0 / (ROPE_BASE ** (np.arange(0, half, dtype=np.float32) * 2.0 / HD))
    ).astype(np.float32)
    ang = pos.astype(np.float32)[:, None] * inv_freq[None, :]  # [S, 64]
    cos = np.cos(ang).astype(np.float32)
    sin = np.sin(ang).astype(np.float32)

    def tab(w):
        w1, w2 = w[:half][None, :], w[half:][None, :]
        return np.ascontiguousarray(
            np.concatenate([cos * w1, sin * w1, cos * w2, sin * w2], axis=1)
        ).astype(np.float32)  # [S, 256] = [cA|sA|cB|sB]

    tq = tab(qw)
    tk = tab(kw)

    wqT = np.ascontiguousarray(wq.T).astype(_BF)
    woT = np.ascontiguousarray(wo.T).astype(_BF)
    # per-half K/V weight slices: even cores own kv heads 0-3, odd 4-7
    wkvT_half = []
    for kvh in range(2):
        rows = slice(kvh * 512, (kvh + 1) * 512)
        wkvT_half.append(
            np.ascontiguousarray(np.concatenate([wk[rows], wv[rows]], axis=0).T).astype(_BF)
        )

    in_maps = []
    for core in range(N_CORES):
        b, qh = core // 2, core % 2
        hsb = np.ascontiguousarray(hs[b].T).astype(_BF)  # [4096, 1024]
        # [a*128+p, tt*128+t] -> [tt, p, a, t]
        hkv = np.ascontiguousarray(
            hsb.reshape(NDT, 128, 8, 128).transpose(2, 1, 0, 3)
        )
        hq = np.ascontiguousarray(
            hsb[:, qh * SQ:(qh + 1) * SQ].reshape(NDT, 128, 4, 128).transpose(2, 1, 0, 3)
        )
        in_maps.append(
            dict(
                hs_kv=hkv,
                hs_q=hq,
                wkvT=wkvT_half[qh],
                wqT=wqT,
                woT=woT,
                ropeq=np.ascontiguousarray(tq[qh * SQ:(qh + 1) * SQ]),
                ropek=tk,
            )
        )
    return in_maps


_NC_CACHE = {}


def _get_nc():
    if "nc" not in _NC_CACHE:
        _NC_CACHE["nc"] = build_bass()
    return _NC_CACHE["nc"]


def _run(inputs, **spmd_kwargs):
    nc = _get_nc()
    in_maps = _prep_inputs(inputs)
    res = run_bass_kernel_spmd(nc, in_maps, list(range(N_CORES)), **spmd_kwargs)
    out = np.empty((B, S, HIDDEN), dtype=np.float32)
    for core in range(N_CORES):
        b, qh = core // 2, core % 2
        out[b, qh * SQ:(qh + 1) * SQ, :] = res.results[core]["y"]
    return out, res


def kernel(**inputs) -> np.ndarray:
    out, _ = _run(inputs)
    return out


if __name__ == "__main__":
    nc = build_bass()
    print("built OK:", len(nc.m.functions[0].blocks), "blocks")


# revision 24
# speedup vs baseline: 1.0358x; 1.0018x over previous
"""Trainium2 Bass kernel: GQA attention block (QKV proj + RMSNorm + RoPE +
bidirectional attention + output proj), 8-way parallel.

Sharding: 8 cores = 4 batches x 2 query-token halves. K/V projection work is
deduplicated across the two cores of a batch: each core computes K/V for only
4 of the 8 kv heads (even core: kv 0-3, odd core: kv 4-7, selected by its
wkvT input slice) and the halves are exchanged with a pairwise AllGather
(replica groups [0,1],[2,3],[4,5],[6,7]), overlapped with the Q projection.
Each core then runs attention + o_proj for its 512 query tokens. Host gathers
the 8 output shards.

Per-core kernel (all matmuls in bf16, fp32 accumulation):
  P1  K proj (4 kv heads, all 1024 tokens) -> RMSNorm+RoPE -> PE-transpose
      -> DRAM -> AllGather -> ktT [d, kt, kv, t] (all 8 kv heads)
      V proj likewise (no norm/rope) -> v_all [t, tt, kv, d]
  P2  Q projection + attention, software-pipelined per head: chunk c+1's
      projection matmuls are interleaved with chunk c's score matmuls and
      the previous head's AV matmuls, so the PE never waits on the ScalarE
      exp tail or the GpSimd softmax-denominator reduction.
  P3  o_proj, a-major weight streaming: y [t, o] = aT.T @ woT, fp32 out
"""

import os
import sys
from contextlib import ExitStack

for _p in (
    "/root/.axon_site",
    "/root/.axon_site/_ro/trn_rl_repo",
    "/root/.axon_site/_ro/pypackages",
    "/opt/trn_rl_repo",
):
    if os.path.isdir(_p) and _p not in sys.path:
        sys.path.append(_p)

import ml_dtypes
import numpy as np

import concourse.bacc as bacc
import concourse.bass as bass
import concourse.tile as tile
from concourse import bass_isa, mybir
from concourse.bass_utils import run_bass_kernel_spmd
from concourse.masks import make_identity

BF16 = mybir.dt.bfloat16
F32 = mybir.dt.float32
AF = mybir.ActivationFunctionType
OP = mybir.AluOpType
AX = mybir.AxisListType

B = 4
S = 1024
SQ = 512            # query tokens per core
HIDDEN = 4096
NH = 32
NKV = 8
HD = 128
EPS = 1e-6
ROPE_BASE = 1000000.0
SCALE = float(HD) ** -0.5
NDT = HIDDEN // 128  # 32 contraction tiles
N_CORES = 8
PAIRS = [[0, 1], [2, 3], [4, 5], [6, 7]]

_BF = ml_dtypes.bfloat16


def _bcast_mid(ap, n):
    """[P, X...] -> [P, n, X...] with a stride-0 middle dim."""
    return bass.AP(tensor=ap.tensor, offset=ap.offset, ap=[ap.ap[0], [0, n], *ap.ap[1:]])


def build_bass() -> bass.Bass:
    nc = bacc.Bacc("TRN2", target_bir_lowering=False, debug=False, num_devices=N_CORES)

    # DRAM I/O (per core). hs blocks pre-arranged on host as [tile, p, a, t]
    # so each DMA is one contiguous 1MB read.
    hs_kv = nc.declare_dram_parameter("hs_kv", [8, 128, NDT, 128], BF16, isOutput=False)
    hs_q = nc.declare_dram_parameter("hs_q", [4, 128, NDT, 128], BF16, isOutput=False)
    wkvT = nc.declare_dram_parameter("wkvT", [HIDDEN, 1024], BF16, isOutput=False)
    wqT = nc.declare_dram_parameter("wqT", [HIDDEN, HIDDEN], BF16, isOutput=False)
    woT = nc.declare_dram_parameter("woT", [HIDDEN, HIDDEN], BF16, isOutput=False)
    # rope tables [t, cA|sA|cB|sB] (cos/sin with rms-norm weight folded in)
    ropeq = nc.declare_dram_parameter("ropeq", [SQ, 256], F32, isOutput=False)
    ropek = nc.declare_dram_parameter("ropek", [S, 256], F32, isOutput=False)
    y = nc.declare_dram_parameter("y", [SQ, HIDDEN], F32, isOutput=True)

    with ExitStack() as ctx:
        tc = ctx.enter_context(tile.TileContext(nc))

        persist = ctx.enter_context(tc.tile_pool(name="persist", bufs=1))
        ktT = persist.tile([128, 8, NKV, 128], BF16, tag="ktT")   # [d, kt, kvh, t]
        v_all = persist.tile([128, 8, NKV, 128], BF16, tag="v")   # [t%128, tt, kvh, d]
        aT = persist.tile([128, NH, SQ], BF16, tag="aT")          # [d, h, q]
        tabq = persist.tile([128, 4, 256], F32, tag="tabq")
        tabk = persist.tile([128, 8, 256], F32, tag="tabk")
        ident = persist.tile([128, 128], BF16, tag="ident")

        wp = ctx.enter_context(tc.tile_pool(name="wp", bufs=12))
        hp = ctx.enter_context(tc.tile_pool(name="hp", bufs=3))
        scratch = ctx.enter_context(tc.tile_pool(name="scratch", bufs=2))
        qtp = ctx.enter_context(tc.tile_pool(name="qtp", bufs=1))
        qnp = ctx.enter_context(tc.tile_pool(name="qnp", bufs=6))
        attn_sb = ctx.enter_context(tc.tile_pool(name="attn_sb", bufs=2))
        ysb = ctx.enter_context(tc.tile_pool(name="ysb", bufs=1))

        dram = ctx.enter_context(tc.tile_pool(name="dram", bufs=1, space="DRAM"))
        cc_kin = dram.tile([128, 8, 4, 128], BF16, tag="cc_kin")
        cc_kout = dram.tile([2, 128, 8, 4, 128], BF16, tag="cc_kout")
        cc_vin = dram.tile([128, 8, 4, 128], BF16, tag="cc_vin")
        cc_vout = dram.tile([2, 128, 8, 4, 128], BF16, tag="cc_vout")

        pp_ps = ctx.enter_context(tc.tile_pool(name="pp_ps", bufs=3, space="PSUM"))
        st_ps = ctx.enter_context(tc.tile_pool(name="st_ps", bufs=3, space="PSUM"))
        av_ps = ctx.enter_context(tc.tile_pool(name="av_ps", bufs=2, space="PSUM"))

        def load_w_group(wsrc, col0, g):
            """One [128, 4, 512] tile covering rows g*512..g*512+512 (4
            contraction tiles), cols col0:col0+512 — a single 512KB DMA, so
            4 consecutive matmuls share one producer dependency."""
            wt = wp.tile([128, 4, 512], BF16, tag="wt")
            nc.sync.dma_start(
                out=wt[:],
                in_=wsrc[g * 512:(g + 1) * 512, col0:col0 + 512].rearrange(
                    "(a p) c -> p a c", p=128),
            )
            return wt

        def load_w_tiles(wsrc, col0):
            return [load_w_group(wsrc, col0, g) for g in range(8)]

        def wslice(wts, a):
            return wts[a // 4][:, a % 4, :]

        def load_hs(src):
            """[128, 32, 128] hidden-state tile, split into 4 sub-DMAs so
            early matmuls only wait on their own quarter."""
            hs_cb = hp.tile([128, NDT, 128], BF16, tag="hs")
            for part in range(4):
                nc.sync.dma_start(out=hs_cb[:, part * 8:(part + 1) * 8, :],
                                  in_=src[:, part * 8:(part + 1) * 8, :])
            return hs_cb

        # First weight group + first hs quarters first: nothing blocks the
        # PE longer than these at kernel start.
        wts_k = [load_w_group(wkvT, 0, 0)]
        hs_first = load_hs(hs_kv[0])
        for g in range(1, 8):
            wts_k.append(load_w_group(wkvT, 0, g))
        make_identity(nc, ident[:])
        nc.sync.dma_start(out=tabq[:], in_=ropeq[:].rearrange("(a p) c -> p a c", p=128))
        nc.sync.dma_start(out=tabk[:], in_=ropek[:].rearrange("(a p) c -> p a c", p=128))

        def norm_rope(ps, tab_tile, tt, qn):
            """RMSNorm + RoPE on a [128 tok, 4 heads, 128] psum projection,
            into bf16 qn [128, 4, 128]."""
            psv = ps[:].rearrange("p (h d) -> p h d", h=4)
            qf = scratch.tile([128, 4, 128], F32, tag="qf")
            qsq = scratch.tile([128, 512], BF16, tag="qsq", bufs=1)
            ssq = scratch.tile([128, 4], F32, tag="ssq")
            rr = scratch.tile([128, 4], F32, tag="rr")
            t1 = scratch.tile([128, 4, 64], F32, tag="tq")
            t2 = scratch.tile([128, 4, 64], F32, tag="tq")
            t3 = scratch.tile([128, 4, 64], F32, tag="tq", name="t3")
            t4 = scratch.tile([128, 4, 64], F32, tag="tq", name="t4")

            nc.scalar.copy(out=qf[:], in_=psv)
            nc.scalar.activation(out=qsq[:], in_=ps[:], func=AF.Square)
            nc.vector.reduce_sum(
                out=ssq[:], in_=qsq[:].rearrange("p (h d) -> p h d", h=4), axis=AX.X
            )
            # v = ssq/128 + eps, then r = rsqrt(v) via bit-trick seed + 2 Newton
            # iterations (all-DVE; keeps ScalarE on a single ACT table set).
            vv = scratch.tile([128, 4], F32, tag="vv")
            rt = scratch.tile([128, 4], F32, tag="rt")
            nc.vector.tensor_scalar(out=vv[:], in0=ssq[:], scalar1=1.0 / HD,
                                    scalar2=EPS, op0=OP.mult, op1=OP.add)
            vi = vv[:].bitcast(mybir.dt.int32)
            ri = rr[:].bitcast(mybir.dt.int32)
            nc.vector.tensor_scalar(out=ri, in0=vi, scalar1=1, scalar2=None,
                                    op0=OP.arith_shift_right)
            nc.vector.tensor_scalar(out=ri, in0=ri, scalar1=-1, scalar2=0x5F3759DF,
                                    op0=OP.mult, op1=OP.add)
            for _ in range(2):
                nc.vector.tensor_mul(rt[:], rr[:], rr[:])
                nc.vector.tensor_mul(rt[:], rt[:], vv[:])
                nc.vector.tensor_scalar(out=rt[:], in0=rt[:], scalar1=-0.5,
                                        scalar2=1.5, op0=OP.mult, op1=OP.add)
                nc.vector.tensor_mul(rr[:], rr[:], rt[:])
            rv = rr[:]
            rr_b = bass.AP(tensor=rv.tensor, offset=rv.offset,
                           ap=[rv.ap[0], rv.ap[1], [0, 128]])
            nc.vector.tensor_mul(qf[:], qf[:], rr_b)
            q1 = qf[:, :, 0:64]
            q2 = qf[:, :, 64:128]
            cA = _bcast_mid(tab_tile[:, tt, 0:64], 4)
            sA = _bcast_mid(tab_tile[:, tt, 64:128], 4)
            cB = _bcast_mid(tab_tile[:, tt, 128:192], 4)
            sB = _bcast_mid(tab_tile[:, tt, 192:256], 4)
            nc.vector.tensor_mul(t1[:], q1, cA)
            nc.vector.tensor_mul(t2[:], q2, sB)
            nc.vector.tensor_sub(qn[:, :, 0:64], t1[:], t2[:])
            nc.vector.tensor_mul(t3[:], q2, cB)
            nc.vector.tensor_mul(t4[:], q1, sA)
            nc.vector.tensor_add(qn[:, :, 64:128], t3[:], t4[:])

        def transpose4(qn, dst_ap):
            """PE-transpose 4 [128,128] heads of qn into dst_ap [128, 4, 128]."""
            tp = st_ps.tile([128, 512], BF16, tag="misc")
            for hh in range(4):
                nc.tensor.transpose(tp[:, hh * 128:(hh + 1) * 128], qn[:, hh, :], ident[:])
            nc.scalar.copy(out=dst_ap, in_=tp[:].rearrange("p (h t) -> p h t", h=4))

        # ---------------- P1: K then V projection (own 4 kv heads) -----
        # K transposes are deferred one tile behind the matmul stream so
        # the PE never waits for the DVE norm/rope tail. The K exchange is
        # split in two so the first AllGather fires ~halfway through the K
        # pass; bounce-buffer DMAs ride the ScalarE queue (SyncE backlogs).
        def flush_k(kn, tt):
            ktn = qnp.tile([128, 4, 128], BF16, tag="qqn", name="ktn")
            transpose4(kn, ktn[:])
            nc.scalar.dma_start(out=cc_kin[:, tt], in_=ktn[:])

        pend_k = None
        for tt in range(8):
            hs_cb = hs_first if tt == 0 else load_hs(hs_kv[tt])
            ps = pp_ps.tile([128, 512], F32, tag="pp")
            for a in range(NDT):
                nc.tensor.matmul(
                    ps[:], hs_cb[:, a, :], wslice(wts_k, a),
                    start=(a == 0), stop=(a == NDT - 1),
                )
            kn = qnp.tile([128, 4, 128], BF16, tag="qqn")
            norm_rope(ps, tabk, tt, kn)
            if pend_k is not None:
                flush_k(*pend_k)
            pend_k = (kn, tt)
        flush_k(*pend_k)
        nc.gpsimd.collective_compute(
            "AllGather", OP.bypass, replica_groups=PAIRS,
            ins=[cc_kin[:]], outs=[cc_kout[:]],
        )
        for half in range(2):
            ts = slice(half * 4, half * 4 + 4)
            nc.gpsimd.dma_start(out=ktT[:, ts, 0:4, :], in_=cc_kout[0][:, ts])
            nc.gpsimd.dma_start(out=ktT[:, ts, 4:8, :], in_=cc_kout[1][:, ts])

        wts_v = load_w_tiles(wkvT, 512)
        wts_q0 = None
        for tt in range(8):
            hs_cb = load_hs(hs_kv[tt])
            if tt == 6:
                wts_q0 = load_w_tiles(wqT, 0)
            ps = pp_ps.tile([128, 512], F32, tag="pp")
            for a in range(NDT):
                nc.tensor.matmul(
                    ps[:], hs_cb[:, a, :], wslice(wts_v, a),
                    start=(a == 0), stop=(a == NDT - 1),
                )
            vn = qnp.tile([128, 4, 128], BF16, tag="vn", name="vn", bufs=2)
            nc.scalar.copy(out=vn[:], in_=ps[:].rearrange("p (h d) -> p h d", h=4))
            nc.scalar.dma_start(out=cc_vin[:, tt], in_=vn[:])
        nc.gpsimd.collective_compute(
            "AllGather", OP.bypass, replica_groups=PAIRS,
            ins=[cc_vin[:]], outs=[cc_vout[:]],
        )
        for half in range(2):
            ts = slice(half * 4, half * 4 + 4)
            nc.gpsimd.dma_start(out=v_all[:, ts, 0:4, :], in_=cc_vout[0][:, ts])
            nc.gpsimd.dma_start(out=v_all[:, ts, 4:8, :], in_=cc_vout[1][:, ts])

        # ---------------- P2: Q projection + attention, pipelined ------
        # Per chunk c (4 q heads sharing kv head c): chunk c+1's
        # projection matmuls interleave with chunk c's score matmuls and
        # older heads' AV matmuls (a 2-deep pending queue), so exp
        # (ScalarE) and the softmax denominator (DVE+GpSimd) latency never
        # stalls the PE. Projections run as token-tile PAIRS sharing each
        # weight tile between two back-to-back matmuls: the second issue
        # skips the per-tile sync overhead the PE pays on a fresh tile.
        def load_hs_q(src_ap):
            hs_cb = hp.tile([128, NDT, 128], BF16, tag="hs")
            for part in range(2):
                nc.sync.dma_start(out=hs_cb[:, part * 16:(part + 1) * 16, :],
                                  in_=src_ap[:, part * 16:(part + 1) * 16, :])
            return hs_cb

        def emit_proj_pair(qt0, wts):
            hs0 = load_hs_q(hs_q[qt0])
            hs1 = load_hs_q(hs_q[qt0 + 1])
            ps0 = pp_ps.tile([128, 512], F32, tag="pp")
            ps1 = pp_ps.tile([128, 512], F32, tag="pp")
            for a in range(NDT):
                nc.tensor.matmul(ps0[:], hs0[:, a, :], wslice(wts, a),
                                 start=(a == 0), stop=(a == NDT - 1))
                nc.tensor.matmul(ps1[:], hs1[:, a, :], wslice(wts, a),
                                 start=(a == 0), stop=(a == NDT - 1))
            qn0 = qnp.tile([128, 4, 128], BF16, tag="qqn")
            norm_rope(ps0, tabq, qt0, qn0)
            qn1 = qnp.tile([128, 4, 128], BF16, tag="qqn")
            norm_rope(ps1, tabq, qt0 + 1, qn1)
            return [qn0, qn1]

        def emit_scores(c, qt, qTc):
            p_sb = attn_sb.tile([128, 8, 512], BF16, tag="p_sb", bufs=3)
            for kt in range(8):
                st = st_ps.tile([128, 512], F32, tag="misc")
                nc.tensor.matmul(
                    st[:], ktT[:, kt, c, :], qTc[:, qt, :],
                    start=True, stop=True,
                )
                nc.scalar.activation(out=p_sb[:, kt, :], in_=st[:],
                                     func=AF.Exp, scale=SCALE)
            # softmax denominator: sum p over the 8 key blocks (DVE), then
            # across partitions (GpSimd all-reduce), then 1/Z (DVE approx)
            acc = attn_sb.tile([128, 512], BF16, tag="acc")
            s01 = attn_sb.tile([128, 512], BF16, tag="s01", bufs=1)
            s23 = attn_sb.tile([128, 512], BF16, tag="s23", bufs=1)
            nc.vector.tensor_add(s01[:], p_sb[:, 0, :], p_sb[:, 1, :])
            nc.vector.tensor_add(s23[:], p_sb[:, 2, :], p_sb[:, 3, :])
            nc.vector.tensor_add(s01[:], s01[:], s23[:])
            nc.vector.tensor_add(acc[:], p_sb[:, 4, :], p_sb[:, 5, :])
            nc.vector.tensor_add(s23[:], p_sb[:, 6, :], p_sb[:, 7, :])
            nc.vector.tensor_add(acc[:], acc[:], s23[:])
            nc.vector.tensor_add(acc[:], acc[:], s01[:])
            zbc = attn_sb.tile([128, 512], F32, tag="zbc", bufs=1)
            nc.gpsimd.partition_all_reduce(out_ap=zbc[:], in_ap=acc[:],
                                           channels=128,
                                           reduce_op=bass_isa.ReduceOp.add)
            rz = attn_sb.tile([128, 512], F32, tag="rz", bufs=2)
            nc.vector.reciprocal_approx_fast(out=rz[:], in_=zbc[:])
            return (c, qt, p_sb, rz)

        def emit_avchain(pend):
            c, qt, p_sb, rz = pend
            h = c * 4 + qt
            av = av_ps.tile([128, 512], F32, tag="av")
            for kt in range(8):
                nc.tensor.matmul(
                    av[:], v_all[:, kt, c, :], p_sb[:, kt, :],
                    start=(kt == 0), stop=(kt == 7),
                )
            nc.vector.tensor_mul(aT[:, h, :], av[:], rz[:])

        from collections import deque
        pend = deque()

        def emit_sc_av(c, qt, qTc):
            if len(pend) >= 2:
                emit_avchain(pend.popleft())
            pend.append(emit_scores(c, qt, qTc))

        wts_q = wts_q0
        qn_next = emit_proj_pair(0, wts_q) + emit_proj_pair(2, wts_q)
        for c in range(8):
            qns = qn_next
            qTc = qtp.tile([128, 4, SQ], BF16, tag="qTc")
            for qt in range(4):
                transpose4(qns[qt], qTc[:, :, qt * 128:(qt + 1) * 128])
            if c < 7:
                wts_q = load_w_tiles(wqT, (c + 1) * 512)
                qn_next = emit_proj_pair(0, wts_q)
            emit_sc_av(c, 0, qTc)
            if c < 7:
                qn_next = qn_next + emit_proj_pair(2, wts_q)
            emit_sc_av(c, 1, qTc)
            emit_sc_av(c, 2, qTc)
            emit_sc_av(c, 3, qTc)
        while pend:
            emit_avchain(pend.popleft())

        # ---------------- P3: o_proj, a-major weight streaming ----------
        # Each weight tile is read by 4 back-to-back matmuls (one per query
        # block) and retires immediately, so the wp ring gives a deep DMA
        # prefetch runway and weights stream continuously. PSUM tiles
        # rotate through the pp/av/st pools (6 banks) so no pool is closed
        # or opened (pool boundaries act as barriers) and chunk boundaries
        # never wait on the previous chunk's PSUM evictions.
        ops_pools = [pp_ps, pp_ps, pp_ps, av_ps, av_ps, st_ps, st_ps]
        ops_tags = ["pp", "pp", "pp", "av", "av", "misc", "misc"]
        ops_idx = 0
        for c in range(8):
            pss = []
            for i in range(4):
                pool = ops_pools[ops_idx % 7]
                pss.append(pool.tile([128, 512], F32, tag=ops_tags[ops_idx % 7],
                                     name=f"ops_{c}_{i}"))
                ops_idx += 1
            for a in range(NDT):
                if a % 4 == 0:
                    wt = load_w_group(woT, c * 512, a // 4)
                for qt in range(4):
                    nc.tensor.matmul(
                        pss[qt][:], aT[:, a, qt * 128:(qt + 1) * 128], wt[:, a % 4, :],
                        start=(a == 0), stop=(a == NDT - 1),
                    )
            for qt in range(4):
                yt = ysb.tile([128, 512], F32, tag="yt")
                if qt % 2 == 0:
                    nc.scalar.copy(out=yt[:], in_=pss[qt][:])
                else:
                    nc.vector.tensor_scalar_mul(yt[:], pss[qt][:], 1.0)
                nc.sync.dma_start(
                    out=y[qt * 128:(qt + 1) * 128, c * 512:(c + 1) * 512], in_=yt[:]
                )

    nc.finalize()
    return nc


def _prep_inputs(inputs):
    pos = np.asarray(inputs["positions"]).astype(np.int32)
    hs = np.asarray(inputs["hidden_states"], dtype=np.float32)
    wq = np.asarray(inputs["wq"], dtype=np.float32)
    wk = np.asarray(inputs["wk"], dtype=np.float32)
    wv = np.asarray(inputs["wv"], dtype=np.float32)
    wo = np.asarray(inputs["wo"], dtype=np.float32)
    qw = np.asarray(inputs["q_norm_w"], dtype=np.float32)
    kw = np.asarray(inputs["k_norm_w"], dtype=np.float32)

    half = HD // 2
    inv_freq = (
        1.0 / (ROPE_BASE ** (np.arange(0, half, dtype=np.float32) * 2.0 / HD))
    ).astype(np.float32)
    ang = pos.astype(np.float32)[:, None] * inv_freq[None, :]  # [S, 64]
    cos = np.cos(ang).astype(np.float32)
    sin = np.sin(ang).astype(np.float32)

    def tab(w):
        w1, w2 = w[:half][None, :], w[half:][None, :]
        return np.ascontiguousarray(
            np.concatenate([cos * w1, sin * w1, cos * w2, sin * w2], axis=1)
        ).astype(np.float32)  # [S, 256] = [cA|sA|cB|sB]

    tq = tab(qw)
    tk = tab(kw)

    wqT = np.ascontiguousarray(wq.T).astype(_BF)
    woT = np.ascontiguousarray(wo.T).astype(_BF)
    # per-half K/V weight slices: even cores own kv heads 0-3, odd 4-7
    wkvT_half = []
    for kvh in range(2):
        rows = slice(kvh * 512, (kvh + 1) * 512)
        wkvT_half.append(
            np.ascontiguousarray(np.concatenate([wk[rows], wv[rows]], axis=0).T).astype(_BF)
        )

    in_maps = []
    for core in range(N_CORES):
        b, qh = core // 2, core % 2
        hsb = np.ascontiguousarray(hs[b].T).astype(_BF)  # [4096, 1024]
        # [a*128+p, tt*128+t] -> [tt, p, a, t]
        hkv = np.ascontiguousarray(
            hsb.reshape(NDT, 128, 8, 128).transpose(2, 1, 0, 3)
        )
        hq = np.ascontiguousarray(
            hsb[:, qh * SQ:(qh + 1) * SQ].reshape(NDT, 128, 4, 128).transpose(2, 1, 0, 3)
        )
        in_maps.append(
            dict(
                hs_kv=hkv,
                hs_q=hq,
                wkvT=wkvT_half[qh],
                wqT=wqT,
                woT=woT,
                ropeq=np.ascontiguousarray(tq[qh * SQ:(qh + 1) * SQ]),
                ropek=tk,
            )
        )
    return in_maps


_NC_CACHE = {}


def _get_nc():
    if "nc" not in _NC_CACHE:
        _NC_CACHE["nc"] = build_bass()
    return _NC_CACHE["nc"]


def _run(inputs, **spmd_kwargs):
    nc = _get_nc()
    in_maps = _prep_inputs(inputs)
    res = run_bass_kernel_spmd(nc, in_maps, list(range(N_CORES)), **spmd_kwargs)
    out = np.empty((B, S, HIDDEN), dtype=np.float32)
    for core in range(N_CORES):
        b, qh = core // 2, core % 2
        out[b, qh * SQ:(qh + 1) * SQ, :] = res.results[core]["y"]
    return out, res


def kernel(**inputs) -> np.ndarray:
    out, _ = _run(inputs)
    return out


if __name__ == "__main__":
    nc = build_bass()
    print("built OK:", len(nc.m.functions[0].blocks), "blocks")


# revision 25
# speedup vs baseline: 1.0475x; 1.0113x over previous
"""Trainium2 Bass kernel: GQA attention block (QKV proj + RMSNorm + RoPE +
bidirectional attention + output proj), 8-way parallel.

Sharding: 8 cores = 4 batches x 2 query-token halves. K/V projection work is
deduplicated across the two cores of a batch: each core computes K/V for only
4 of the 8 kv heads (even core: kv 0-3, odd core: kv 4-7, selected by its
wkvT input slice) and the halves are exchanged with a pairwise AllGather
(replica groups [0,1],[2,3],[4,5],[6,7]), overlapped with the Q projection.
Each core then runs attention + o_proj for its 512 query tokens. Host gathers
the 8 output shards.

Per-core kernel (all matmuls in bf16, fp32 accumulation):
  P1  K proj (4 kv heads, all 1024 tokens) -> RMSNorm+RoPE -> PE-transpose
      -> DRAM -> AllGather -> ktT [d, kt, kv, t] (all 8 kv heads)
      V proj likewise (no norm/rope) -> v_all [t, tt, kv, d]
  P2  Q projection + attention, software-pipelined per head: chunk c+1's
      projection matmuls are interleaved with chunk c's score matmuls and
      the previous head's AV matmuls, so the PE never waits on the ScalarE
      exp tail or the GpSimd softmax-denominator reduction.
  P3  o_proj, a-major weight streaming: y [t, o] = aT.T @ woT, fp32 out
"""

import os
import sys
from contextlib import ExitStack

for _p in (
    "/root/.axon_site",
    "/root/.axon_site/_ro/trn_rl_repo",
    "/root/.axon_site/_ro/pypackages",
    "/opt/trn_rl_repo",
):
    if os.path.isdir(_p) and _p not in sys.path:
        sys.path.append(_p)

import ml_dtypes
import numpy as np

import concourse.bacc as bacc
import concourse.bass as bass
import concourse.tile as tile
from concourse import bass_isa, mybir
from concourse.bass_utils import run_bass_kernel_spmd
from concourse.masks import make_identity

BF16 = mybir.dt.bfloat16
F32 = mybir.dt.float32
AF = mybir.ActivationFunctionType
OP = mybir.AluOpType
AX = mybir.AxisListType

B = 4
S = 1024
SQ = 512            # query tokens per core
HIDDEN = 4096
NH = 32
NKV = 8
HD = 128
EPS = 1e-6
ROPE_BASE = 1000000.0
SCALE = float(HD) ** -0.5
NDT = HIDDEN // 128  # 32 contraction tiles
N_CORES = 8
PAIRS = [[0, 1], [2, 3], [4, 5], [6, 7]]

_BF = ml_dtypes.bfloat16


def _bcast_mid(ap, n):
    """[P, X...] -> [P, n, X...] with a stride-0 middle dim."""
    return bass.AP(tensor=ap.tensor, offset=ap.offset, ap=[ap.ap[0], [0, n], *ap.ap[1:]])


def build_bass() -> bass.Bass:
    nc = bacc.Bacc("TRN2", target_bir_lowering=False, debug=False, num_devices=N_CORES)

    # DRAM I/O (per core). hs blocks pre-arranged on host as [tile, p, a, t]
    # so each DMA is one contiguous 1MB read.
    hs_kv = nc.declare_dram_parameter("hs_kv", [8, 128, NDT, 128], BF16, isOutput=False)
    hs_q = nc.declare_dram_parameter("hs_q", [4, 128, NDT, 128], BF16, isOutput=False)
    wkvT = nc.declare_dram_parameter("wkvT", [HIDDEN, 1024], BF16, isOutput=False)
    wqT = nc.declare_dram_parameter("wqT", [HIDDEN, HIDDEN], BF16, isOutput=False)
    woT = nc.declare_dram_parameter("woT", [HIDDEN, HIDDEN], BF16, isOutput=False)
    # rope tables [t, cA|sA|cB|sB] (cos/sin with rms-norm weight folded in)
    ropeq = nc.declare_dram_parameter("ropeq", [SQ, 256], F32, isOutput=False)
    ropek = nc.declare_dram_parameter("ropek", [S, 256], F32, isOutput=False)
    y = nc.declare_dram_parameter("y", [SQ, HIDDEN], F32, isOutput=True)

    with ExitStack() as ctx:
        tc = ctx.enter_context(tile.TileContext(nc))

        persist = ctx.enter_context(tc.tile_pool(name="persist", bufs=1))
        ktT = persist.tile([128, 8, NKV, 128], BF16, tag="ktT")   # [d, kt, kvh, t]
        v_all = persist.tile([128, 8, NKV, 128], BF16, tag="v")   # [t%128, tt, kvh, d]
        aT = persist.tile([128, NH, SQ], BF16, tag="aT")          # [d, h, q]
        tabq = persist.tile([128, 4, 256], F32, tag="tabq")
        tabk = persist.tile([128, 8, 256], F32, tag="tabk")
        ident = persist.tile([128, 128], BF16, tag="ident")

        wp = ctx.enter_context(tc.tile_pool(name="wp", bufs=10))
        hp = ctx.enter_context(tc.tile_pool(name="hp", bufs=4))
        scratch = ctx.enter_context(tc.tile_pool(name="scratch", bufs=2))
        qtp = ctx.enter_context(tc.tile_pool(name="qtp", bufs=1))
        qnp = ctx.enter_context(tc.tile_pool(name="qnp", bufs=5))
        attn_sb = ctx.enter_context(tc.tile_pool(name="attn_sb", bufs=2))
        ysb = ctx.enter_context(tc.tile_pool(name="ysb", bufs=2))

        dram = ctx.enter_context(tc.tile_pool(name="dram", bufs=1, space="DRAM"))
        cc_kin = dram.tile([128, 8, 4, 128], BF16, tag="cc_kin")
        cc_kout = dram.tile([2, 128, 8, 4, 128], BF16, tag="cc_kout")
        cc_vin = dram.tile([128, 8, 4, 128], BF16, tag="cc_vin")
        cc_vout = dram.tile([2, 128, 8, 4, 128], BF16, tag="cc_vout")

        pp_ps = ctx.enter_context(tc.tile_pool(name="pp_ps", bufs=3, space="PSUM"))
        st_ps = ctx.enter_context(tc.tile_pool(name="st_ps", bufs=3, space="PSUM"))
        av_ps = ctx.enter_context(tc.tile_pool(name="av_ps", bufs=2, space="PSUM"))

        def load_w_group(wsrc, col0, g):
            """One [128, 4, 512] tile covering rows g*512..g*512+512 (4
            contraction tiles), cols col0:col0+512 — a single 512KB DMA, so
            4 consecutive matmuls share one producer dependency."""
            wt = wp.tile([128, 4, 512], BF16, tag="wt")
            nc.sync.dma_start(
                out=wt[:],
                in_=wsrc[g * 512:(g + 1) * 512, col0:col0 + 512].rearrange(
                    "(a p) c -> p a c", p=128),
            )
            return wt

        def load_w_tiles(wsrc, col0):
            return [load_w_group(wsrc, col0, g) for g in range(8)]

        def wslice(wts, a):
            return wts[a // 4][:, a % 4, :]

        def load_hs(src):
            """[128, 32, 128] hidden-state tile, split into 4 sub-DMAs so
            early matmuls only wait on their own quarter."""
            hs_cb = hp.tile([128, NDT, 128], BF16, tag="hs")
            for part in range(4):
                nc.sync.dma_start(out=hs_cb[:, part * 8:(part + 1) * 8, :],
                                  in_=src[:, part * 8:(part + 1) * 8, :])
            return hs_cb

        # First weight group + first hs quarters first: nothing blocks the
        # PE longer than these at kernel start.
        wts_k = [load_w_group(wkvT, 0, 0)]
        hs_first = load_hs(hs_kv[0])
        for g in range(1, 8):
            wts_k.append(load_w_group(wkvT, 0, g))
        make_identity(nc, ident[:])
        nc.sync.dma_start(out=tabq[:], in_=ropeq[:].rearrange("(a p) c -> p a c", p=128))
        nc.sync.dma_start(out=tabk[:], in_=ropek[:].rearrange("(a p) c -> p a c", p=128))

        def norm_rope(ps, tab_tile, tt, qn):
            """RMSNorm + RoPE on a [128 tok, 4 heads, 128] psum projection,
            into bf16 qn [128, 4, 128]."""
            psv = ps[:].rearrange("p (h d) -> p h d", h=4)
            qf = scratch.tile([128, 4, 128], F32, tag="qf")
            qsq = scratch.tile([128, 512], BF16, tag="qsq", bufs=1)
            ssq = scratch.tile([128, 4], F32, tag="ssq")
            rr = scratch.tile([128, 4], F32, tag="rr")
            t1 = scratch.tile([128, 4, 64], F32, tag="tq")
            t2 = scratch.tile([128, 4, 64], F32, tag="tq")
            t3 = scratch.tile([128, 4, 64], F32, tag="tq", name="t3")
            t4 = scratch.tile([128, 4, 64], F32, tag="tq", name="t4")

            nc.scalar.copy(out=qf[:], in_=psv)
            nc.scalar.activation(out=qsq[:], in_=ps[:], func=AF.Square)
            nc.vector.reduce_sum(
                out=ssq[:], in_=qsq[:].rearrange("p (h d) -> p h d", h=4), axis=AX.X
            )
            # v = ssq/128 + eps, then r = rsqrt(v) via bit-trick seed + 2 Newton
            # iterations (all-DVE; keeps ScalarE on a single ACT table set).
            vv = scratch.tile([128, 4], F32, tag="vv")
            rt = scratch.tile([128, 4], F32, tag="rt")
            nc.vector.tensor_scalar(out=vv[:], in0=ssq[:], scalar1=1.0 / HD,
                                    scalar2=EPS, op0=OP.mult, op1=OP.add)
            vi = vv[:].bitcast(mybir.dt.int32)
            ri = rr[:].bitcast(mybir.dt.int32)
            nc.vector.tensor_scalar(out=ri, in0=vi, scalar1=1, scalar2=None,
                                    op0=OP.arith_shift_right)
            nc.vector.tensor_scalar(out=ri, in0=ri, scalar1=-1, scalar2=0x5F3759DF,
                                    op0=OP.mult, op1=OP.add)
            for _ in range(2):
                nc.vector.tensor_mul(rt[:], rr[:], rr[:])
                nc.vector.tensor_mul(rt[:], rt[:], vv[:])
                nc.vector.tensor_scalar(out=rt[:], in0=rt[:], scalar1=-0.5,
                                        scalar2=1.5, op0=OP.mult, op1=OP.add)
                nc.vector.tensor_mul(rr[:], rr[:], rt[:])
            rv = rr[:]
            rr_b = bass.AP(tensor=rv.tensor, offset=rv.offset,
                           ap=[rv.ap[0], rv.ap[1], [0, 128]])
            nc.vector.tensor_mul(qf[:], qf[:], rr_b)
            q1 = qf[:, :, 0:64]
            q2 = qf[:, :, 64:128]
            cA = _bcast_mid(tab_tile[:, tt, 0:64], 4)
            sA = _bcast_mid(tab_tile[:, tt, 64:128], 4)
            cB = _bcast_mid(tab_tile[:, tt, 128:192], 4)
            sB = _bcast_mid(tab_tile[:, tt, 192:256], 4)
            nc.vector.tensor_mul(t1[:], q1, cA)
            nc.vector.tensor_mul(t2[:], q2, sB)
            nc.vector.tensor_sub(qn[:, :, 0:64], t1[:], t2[:])
            nc.vector.tensor_mul(t3[:], q2, cB)
            nc.vector.tensor_mul(t4[:], q1, sA)
            nc.vector.tensor_add(qn[:, :, 64:128], t3[:], t4[:])

        def transpose4(qn, dst_ap):
            """PE-transpose 4 [128,128] heads of qn into dst_ap [128, 4, 128]."""
            tp = st_ps.tile([128, 512], BF16, tag="misc")
            for hh in range(4):
                nc.tensor.transpose(tp[:, hh * 128:(hh + 1) * 128], qn[:, hh, :], ident[:])
            nc.scalar.copy(out=dst_ap, in_=tp[:].rearrange("p (h t) -> p h t", h=4))

        # ---------------- P1: K then V projection (own 4 kv heads) -----
        # K transposes are deferred one tile behind the matmul stream so
        # the PE never waits for the DVE norm/rope tail. The K exchange is
        # split in two so the first AllGather fires ~halfway through the K
        # pass; bounce-buffer DMAs ride the ScalarE queue (SyncE backlogs).
        def flush_k(kn, tt):
            ktn = qnp.tile([128, 4, 128], BF16, tag="qqn", name="ktn")
            transpose4(kn, ktn[:])
            nc.scalar.dma_start(out=cc_kin[:, tt], in_=ktn[:])

        pend_k = None
        for tt in range(8):
            hs_cb = hs_first if tt == 0 else load_hs(hs_kv[tt])
            ps = pp_ps.tile([128, 512], F32, tag="pp")
            for a in range(NDT):
                nc.tensor.matmul(
                    ps[:], hs_cb[:, a, :], wslice(wts_k, a),
                    start=(a == 0), stop=(a == NDT - 1),
                )
            kn = qnp.tile([128, 4, 128], BF16, tag="qqn")
            norm_rope(ps, tabk, tt, kn)
            if pend_k is not None:
                flush_k(*pend_k)
            pend_k = (kn, tt)
        flush_k(*pend_k)
        nc.gpsimd.collective_compute(
            "AllGather", OP.bypass, replica_groups=PAIRS,
            ins=[cc_kin[:]], outs=[cc_kout[:]],
        )
        for half in range(2):
            ts = slice(half * 4, half * 4 + 4)
            nc.gpsimd.dma_start(out=ktT[:, ts, 0:4, :], in_=cc_kout[0][:, ts])
            nc.gpsimd.dma_start(out=ktT[:, ts, 4:8, :], in_=cc_kout[1][:, ts])

        wts_v = load_w_tiles(wkvT, 512)
        wts_q0 = None
        for tt in range(8):
            hs_cb = load_hs(hs_kv[tt])
            if tt == 6:
                wts_q0 = load_w_tiles(wqT, 0)
            ps = pp_ps.tile([128, 512], F32, tag="pp")
            for a in range(NDT):
                nc.tensor.matmul(
                    ps[:], hs_cb[:, a, :], wslice(wts_v, a),
                    start=(a == 0), stop=(a == NDT - 1),
                )
            vn = qnp.tile([128, 4, 128], BF16, tag="vn", name="vn", bufs=1)
            nc.scalar.copy(out=vn[:], in_=ps[:].rearrange("p (h d) -> p h d", h=4))
            nc.scalar.dma_start(out=cc_vin[:, tt], in_=vn[:])
        nc.gpsimd.collective_compute(
            "AllGather", OP.bypass, replica_groups=PAIRS,
            ins=[cc_vin[:]], outs=[cc_vout[:]],
        )
        for half in range(2):
            ts = slice(half * 4, half * 4 + 4)
            nc.gpsimd.dma_start(out=v_all[:, ts, 0:4, :], in_=cc_vout[0][:, ts])
            nc.gpsimd.dma_start(out=v_all[:, ts, 4:8, :], in_=cc_vout[1][:, ts])

        # ---------------- P2: Q projection + attention, pipelined ------
        # Per chunk c (4 q heads sharing kv head c): chunk c+1's
        # projection matmuls interleave with chunk c's score matmuls and
        # older heads' AV matmuls (a 2-deep pending queue), so exp
        # (ScalarE) and the softmax denominator (DVE+GpSimd) latency never
        # stalls the PE. Projections run as token-tile PAIRS sharing each
        # weight tile between two back-to-back matmuls: the second issue
        # skips the per-tile sync overhead the PE pays on a fresh tile.
        def load_hs_q(src_ap):
            hs_cb = hp.tile([128, NDT, 128], BF16, tag="hs")
            for part in range(2):
                nc.sync.dma_start(out=hs_cb[:, part * 16:(part + 1) * 16, :],
                                  in_=src_ap[:, part * 16:(part + 1) * 16, :])
            return hs_cb

        def emit_proj_pair(qt0, wts):
            hs0 = load_hs_q(hs_q[qt0])
            hs1 = load_hs_q(hs_q[qt0 + 1])
            ps0 = pp_ps.tile([128, 512], F32, tag="pp")
            ps1 = pp_ps.tile([128, 512], F32, tag="pp")
            for a in range(NDT):
                nc.tensor.matmul(ps0[:], hs0[:, a, :], wslice(wts, a),
                                 start=(a == 0), stop=(a == NDT - 1))
                nc.tensor.matmul(ps1[:], hs1[:, a, :], wslice(wts, a),
                                 start=(a == 0), stop=(a == NDT - 1))
            qn0 = qnp.tile([128, 4, 128], BF16, tag="qqn")
            norm_rope(ps0, tabq, qt0, qn0)
            qn1 = qnp.tile([128, 4, 128], BF16, tag="qqn")
            norm_rope(ps1, tabq, qt0 + 1, qn1)
            return [qn0, qn1]

        def emit_scores(c, qt, qTc):
            p_sb = attn_sb.tile([128, 8, 512], BF16, tag="p_sb", bufs=3)
            for kt in range(8):
                st = st_ps.tile([128, 512], F32, tag="misc")
                nc.tensor.matmul(
                    st[:], ktT[:, kt, c, :], qTc[:, qt, :],
                    start=True, stop=True,
                )
                nc.scalar.activation(out=p_sb[:, kt, :], in_=st[:],
                                     func=AF.Exp, scale=SCALE)
            # softmax denominator: sum p over the 8 key blocks (DVE), then
            # across partitions (GpSimd all-reduce), then 1/Z (DVE approx)
            acc = attn_sb.tile([128, 512], BF16, tag="acc")
            s01 = attn_sb.tile([128, 512], BF16, tag="s01", bufs=1)
            s23 = attn_sb.tile([128, 512], BF16, tag="s23", bufs=1)
            nc.vector.tensor_add(s01[:], p_sb[:, 0, :], p_sb[:, 1, :])
            nc.vector.tensor_add(s23[:], p_sb[:, 2, :], p_sb[:, 3, :])
            nc.vector.tensor_add(s01[:], s01[:], s23[:])
            nc.vector.tensor_add(acc[:], p_sb[:, 4, :], p_sb[:, 5, :])
            nc.vector.tensor_add(s23[:], p_sb[:, 6, :], p_sb[:, 7, :])
            nc.vector.tensor_add(acc[:], acc[:], s23[:])
            nc.vector.tensor_add(acc[:], acc[:], s01[:])
            zbc = attn_sb.tile([128, 512], F32, tag="zbc", bufs=1)
            nc.gpsimd.partition_all_reduce(out_ap=zbc[:], in_ap=acc[:],
                                           channels=128,
                                           reduce_op=bass_isa.ReduceOp.add)
            rz = attn_sb.tile([128, 512], F32, tag="rz", bufs=2)
            nc.vector.reciprocal_approx_fast(out=rz[:], in_=zbc[:])
            return (c, qt, p_sb, rz)

        def emit_avchain(pend):
            c, qt, p_sb, rz = pend
            h = c * 4 + qt
            av = av_ps.tile([128, 512], F32, tag="av")
            for kt in range(8):
                nc.tensor.matmul(
                    av[:], v_all[:, kt, c, :], p_sb[:, kt, :],
                    start=(kt == 0), stop=(kt == 7),
                )
            nc.vector.tensor_mul(aT[:, h, :], av[:], rz[:])

        from collections import deque
        pend = deque()

        def emit_sc_av(c, qt, qTc):
            if len(pend) >= 2:
                emit_avchain(pend.popleft())
            pend.append(emit_scores(c, qt, qTc))

        wts_q = wts_q0
        qn_next = emit_proj_pair(0, wts_q) + emit_proj_pair(2, wts_q)
        for c in range(8):
            qns = qn_next
            qTc = qtp.tile([128, 4, SQ], BF16, tag="qTc")
            for qt in range(4):
                transpose4(qns[qt], qTc[:, :, qt * 128:(qt + 1) * 128])
            if c < 7:
                wts_q = load_w_tiles(wqT, (c + 1) * 512)
                qn_next = emit_proj_pair(0, wts_q)
            emit_sc_av(c, 0, qTc)
            if c < 7:
                qn_next = qn_next + emit_proj_pair(2, wts_q)
            emit_sc_av(c, 1, qTc)
            emit_sc_av(c, 2, qTc)
            emit_sc_av(c, 3, qTc)
        while pend:
            emit_avchain(pend.popleft())

        # ---------------- P3: o_proj, a-major weight streaming ----------
        # Each weight tile is read by 4 back-to-back matmuls (one per query
        # block) and retires immediately, so the wp ring gives a deep DMA
        # prefetch runway and weights stream continuously. PSUM tiles
        # rotate through the pp/av/st pools (6 banks) so no pool is closed
        # or opened (pool boundaries act as barriers) and chunk boundaries
        # never wait on the previous chunk's PSUM evictions.
        ops_pools = [pp_ps, pp_ps, pp_ps, av_ps, av_ps, st_ps, st_ps]
        ops_tags = ["pp", "pp", "pp", "av", "av", "misc", "misc"]
        ops_idx = 0
        for c in range(8):
            pss = []
            for i in range(4):
                pool = ops_pools[ops_idx % 7]
                pss.append(pool.tile([128, 512], F32, tag=ops_tags[ops_idx % 7],
                                     name=f"ops_{c}_{i}"))
                ops_idx += 1
            for a in range(NDT):
                if a % 4 == 0:
                    wt = load_w_group(woT, c * 512, a // 4)
                for qt in range(4):
                    nc.tensor.matmul(
                        pss[qt][:], aT[:, a, qt * 128:(qt + 1) * 128], wt[:, a % 4, :],
                        start=(a == 0), stop=(a == NDT - 1),
                    )
            for qt in range(4):
                yt = ysb.tile([128, 512], F32, tag="yt")
                if qt % 2 == 0:
                    nc.scalar.copy(out=yt[:], in_=pss[qt][:])
                else:
                    nc.vector.tensor_scalar_mul(yt[:], pss[qt][:], 1.0)
                nc.sync.dma_start(
                    out=y[qt * 128:(qt + 1) * 128, c * 512:(c + 1) * 512], in_=yt[:]
                )

    nc.finalize()
    return nc


def _prep_inputs(inputs):
    pos = np.asarray(inputs["positions"]).astype(np.int32)
    hs = np.asarray(inputs["hidden_states"], dtype=np.float32)
    wq = np.asarray(inputs["wq"], dtype=np.float32)
    wk = np.asarray(inputs["wk"], dtype=np.float32)
    wv = np.asarray(inputs["wv"], dtype=np.float32)
    wo = np.asarray(inputs["wo"], dtype=np.float32)
    qw = np.asarray(inputs["q_norm_w"], dtype=np.float32)
    kw = np.asarray(inputs["k_norm_w"], dtype=np.float32)

    half = HD // 2
    inv_freq = (
        1.0 / (ROPE_BASE ** (np.arange(0, half, dtype=np.float32) * 2.0 / HD))
    ).astype(np.float32)
    ang = pos.astype(np.float32)[:, None] * inv_freq[None, :]  # [S, 64]
    cos = np.cos(ang).astype(np.float32)
    sin = np.sin(ang).astype(np.float32)

    def tab(w):
        w1, w2 = w[:half][None, :], w[half:][None, :]
        return np.ascontiguousarray(
            np.concatenate([cos * w1, sin * w1, cos * w2, sin * w2], axis=1)
        ).astype(np.float32)  # [S, 256] = [cA|sA|cB|sB]

    tq = tab(qw)
    tk = tab(kw)

    wqT = np.ascontiguousarray(wq.T).astype(_BF)
    woT = np.ascontiguousarray(wo.T).astype(_BF)
    # per-half K/V weight slices: even cores own kv heads 0-3, odd 4-7
    wkvT_half = []
    for kvh in range(2):
        rows = slice(kvh * 512, (kvh + 1) * 512)
        wkvT_half.append(
            np.ascontiguousarray(np.concatenate([wk[rows], wv[rows]], axis=0).T).astype(_BF)
        )

    in_maps = []
    for core in range(N_CORES):
        b, qh = core // 2, core % 2
        hsb = np.ascontiguousarray(hs[b].T).astype(_BF)  # [4096, 1024]
        # [a*128+p, tt*128+t] -> [tt, p, a, t]
        hkv = np.ascontiguousarray(
            hsb.reshape(NDT, 128, 8, 128).transpose(2, 1, 0, 3)
        )
        hq = np.ascontiguousarray(
            hsb[:, qh * SQ:(qh + 1) * SQ].reshape(NDT, 128, 4, 128).transpose(2, 1, 0, 3)
        )
        in_maps.append(
            dict(
                hs_kv=hkv,
                hs_q=hq,
                wkvT=wkvT_half[qh],
                wqT=wqT,
                woT=woT,
                ropeq=np.ascontiguousarray(tq[qh * SQ:(qh + 1) * SQ]),
                ropek=tk,
            )
        )
    return in_maps


_NC_CACHE = {}


def _get_nc():
    if "nc" not in _NC_CACHE:
        _NC_CACHE["nc"] = build_bass()
    return _NC_CACHE["nc"]


def _run(inputs, **spmd_kwargs):
    nc = _get_nc()
    in_maps = _prep_inputs(inputs)
    res = run_bass_kernel_spmd(nc, in_maps, list(range(N_CORES)), **spmd_kwargs)
    out = np.empty((B, S, HIDDEN), dtype=np.float32)
    for core in range(N_CORES):
        b, qh = core // 2, core % 2
        out[b, qh * SQ:(qh + 1) * SQ, :] = res.results[core]["y"]
    return out, res


def kernel(**inputs) -> np.ndarray:
    out, _ = _run(inputs)
    return out


if __name__ == "__main__":
    nc = build_bass()
    print("built OK:", len(nc.m.functions[0].blocks), "blocks")
